# revision 35
# baseline (speedup 1.0000x reference)
"""Trainium2 Bass kernel for nn_EssentialMatrixEstimator.

Distribution: data-parallel over the N=3072 rows of Pc across 8 cores
(384 rows each).

Math: the (N*M, 9) epipolar design-matrix Gram collapses to a single 6x6
monomial Gram C = M1^T W M2 computed in HOST-pre-centered coordinates
x~ = s0*(x - c0) (no cancellation).  The Hartley normalization is a 6x6
linear map on monomials, so the normalized Gram is C2 = L1 C L2^T with
L1/L2 built on-device from the Hartley scalars (which live in row/col 5
of C).  Mmat (9x9) is then a pure index expansion of C2.  One AllGather
(column top-3 partials) + one AllReduce (6x6 Gram) total.
W is the bidirectional-top3 (+ >0.01) masked score matrix; exact top-3
via Max8 (column partials read straight from the transpose PSUM, so the
AllGather posts as soon as the input DMA drains).  Row masking is a
single fused scalar_tensor_tensor pass in row-layout during the gather;
masked data is re-transposed during the same window.  Gram matmuls run
as fp32r with the 6-wide monomial matrix PE-stationary.  The 50-step
power iterations run as rescaled repeated squaring (M <- 2*(M@M)).
Small 3x3/2x3 matrices are assembled from partition-0 scalars via PE
rank-1 (outer-product) matmuls instead of DRAM staging round trips.
"""

import os

os.environ.setdefault("JAX_PLATFORMS", "axon")

import numpy as np

import concourse.bass as bass
import concourse.bass_isa as bass_isa
import concourse.mybir as mybir
import concourse.bacc as bacc
import concourse.tile as tile

NCORES = 8
N = 3072
SH = N // NCORES          # 384 rows per core
RT = SH // 128            # 3 row tiles per core
CB = N // 128             # 24 column blocks
F32 = mybir.dt.float32
F32R = mybir.dt.float32r
AF = mybir.ActivationFunctionType
OP = mybir.AluOpType
AX = mybir.AxisListType

EPS = 1e-8
SQRT2 = 1.4142135623730951
INV_SQRT3 = 1.0 / 1.7320508075688772
T0 = float(np.nextafter(np.float32(0.01), np.float32(1)))  # x > 0.01 == x >= T0
H, W = 64, 64

# host pre-centering of the camera-plane grid coordinates
S0 = 20.0
C0X = -0.577
C0Y = -0.432

# colmask runs entirely on DVE: the Pool/gpsimd engine supports neither
# TensorScalarPtr nor broadcast (stride-0) TensorTensor operands
DVE_CM = CB

# cpack const layout (tensor [9, 48]): column ranges
C_I9H = 0      # I9 * 0.5          [9, 9]
C_ET69 = 9     # E^T selector      [6, 9]
C_I3 = 18      # I3                [3, 3]
C_V09 = 21     # full(1/3)         [9, 1]
C_V06 = 22     # full(1/sqrt3)     [6, 1]
C_SEL1 = 23    # [I3 | 0]          [3, 6]
C_SEL2 = 29    # [0 | I3]          [3, 6]
C_E5 = 35      # e5 selector       [6, 1]
C_I6 = 36      # I6                [6, 6]
C_E6F = 48     # flat I6 rows      [1, 36] (row j at 6j, partition 0)
C_E3F = 84     # flat I3 rows      [1, 9]
C_E2F = 93     # flat I2 rows      [1, 4]

PAIRS = [(0, 0), (0, 1), (0, 2), (1, 1), (1, 2), (2, 2)]


def _pidx():
    d = {}
    for i, (a, b) in enumerate(PAIRS):
        d[(a, b)] = i
        d[(b, a)] = i
    return d


def host_constants(K):
    """Pre-centered monomial matrix + packed tail constants (all f32)."""
    idx = np.arange(H * W, dtype=np.float32)
    pix = np.stack([idx % np.float32(W), np.floor(idx / np.float32(W))], -1)
    K_inv = np.linalg.inv(np.asarray(K, np.float32)).astype(np.float32)
    p1h = np.concatenate([pix[:N], np.ones((N, 1), np.float32)], -1)
    pts = (p1h @ K_inv.T)[:, :2].astype(np.float32)  # same grid both sides
    x = (np.float32(S0) * (pts[:, 0] - np.float32(C0X))).astype(np.float32)
    y = (np.float32(S0) * (pts[:, 1] - np.float32(C0Y))).astype(np.float32)
    M = np.stack([x * x, x * y, x, y * y, y, np.ones_like(x)], -1).astype(np.float32)

    cpack = np.zeros((9, 100), np.float32)
    cpack[:9, C_I9H:C_I9H + 9] = 0.5 * np.eye(9, dtype=np.float32)
    pid = _pidx()
    for a in range(3):
        for b in range(3):
            cpack[pid[(a, b)], C_ET69 + 3 * a + b] = 1.0  # ET69[m, 3a+b]
    cpack[:3, C_I3:C_I3 + 3] = np.eye(3, dtype=np.float32)
    cpack[:9, C_V09] = 1.0 / 3.0
    cpack[:6, C_V06] = INV_SQRT3
    cpack[:3, C_SEL1:C_SEL1 + 3] = np.eye(3, dtype=np.float32)
    cpack[:3, C_SEL2 + 3:C_SEL2 + 6] = np.eye(3, dtype=np.float32)
    cpack[5, C_E5] = 1.0
    cpack[:6, C_I6:C_I6 + 6] = np.eye(6, dtype=np.float32)
    cpack[0, C_E6F:C_E6F + 36] = np.eye(6, dtype=np.float32).reshape(-1)
    cpack[0, C_E3F:C_E3F + 9] = np.eye(3, dtype=np.float32).reshape(-1)
    cpack[0, C_E2F:C_E2F + 4] = np.eye(2, dtype=np.float32).reshape(-1)
    return M, cpack


def _tile128(a, ntiles):
    """[ntiles*128, F] -> [128, ntiles*F] with [p, t*F+f] = a[t*128+p, f]."""
    F = a.shape[1]
    return np.ascontiguousarray(
        a.reshape(ntiles, 128, F).transpose(1, 0, 2).reshape(128, ntiles * F)
    )


def _act_copy(nc, out, in_, scale=1.0):
    nc.scalar.activation(out, in_, AF.Copy, scale=scale)


def _dcp(nc, out, in_, scale=None):
    """Tail copies run on DVE (idle there, lower latency than ACT)."""
    if scale is None:
        nc.vector.tensor_copy(out, in_)
    else:
        nc.vector.tensor_scalar_mul(out, in_, scale)


def build_nc():
    """Build the SPMD 8-core Bass program; returns compiled nc."""
    nc = bacc.Bacc("TRN2", target_bir_lowering=False, debug=False,
                   num_devices=NCORES)

    xin = nc.dram_tensor("xin", [128, RT * N], F32, kind="ExternalInput")
    m1s = nc.dram_tensor("m1s", [128, RT * 6], F32R, kind="ExternalInput")
    m2t = nc.dram_tensor("m2t", [128, CB * 6], F32R, kind="ExternalInput")
    ident = nc.dram_tensor("ident", [128, 128], F32, kind="ExternalInput")
    cpk = nc.dram_tensor("cpack", [9, 100], F32, kind="ExternalInput")
    out_d = nc.dram_tensor("out", [3, 3], F32, kind="ExternalOutput")

    cp_in = nc.dram_tensor("cp_in", [128, CB * 3], F32)
    cp_out = nc.dram_tensor("cp_out", [NCORES * 128, CB * 3], F32,
                            addr_space="Shared")
    cr_in = nc.dram_tensor("cr_in", [6, 6], F32)
    cr_out = nc.dram_tensor("cr_out", [6, 6], F32, addr_space="Shared")
    mshuf = nc.dram_tensor("mshuf", [81], F32)

    groups = [list(range(NCORES))]

    with tile.TileContext(nc) as tc:
        with (
            tc.tile_pool(name="persist", bufs=1) as pp,
            tc.tile_pool(name="scratch", bufs=2) as sp,
            tc.tile_pool(name="ps_pt", bufs=2, space="PSUM") as ps1,
            tc.tile_pool(name="ps_w2", bufs=2, space="PSUM") as psw,
            tc.tile_pool(name="ps_tl", bufs=1, space="PSUM") as ps,
            tc.tile_pool(name="ps_acc", bufs=1, space="PSUM") as psa,
            tc.tile_pool(name="ps_c", bufs=1, space="PSUM") as psc,
        ):
            # ---------- P0: loads ----------
            # idn FIRST: every transpose depends on it, and DMA queues drain
            # in issue order.  One dma + one tile per X row tile, so
            # consumers of tile t wait only chunk t.
            idn = pp.tile([128, 128], F32, tag="idn")
            nc.sync.dma_start(idn[:], ident[:])
            cps = pp.tile([9, 100], F32, tag="cpk")
            nc.sync.dma_start(cps[:], cpk[:])
            Xs = []
            for t in range(RT):
                Xi = pp.tile([128, N], F32, tag=f"X{t}")
                nc.sync.dma_start(Xi[:], xin[:, t * N:(t + 1) * N])
                Xs.append(Xi)
            m1t_s = pp.tile([128, RT * 6], F32R, tag="m1")
            nc.sync.dma_start(m1t_s[:], m1s[:])
            m2t_s = pp.tile([128, CB * 6], F32R, tag="m2")
            nc.sync.dma_start(m2t_s[:], m2t[:])

            def Xt(t):
                return Xs[t][:]

            # ---------- P1: raw transposes -> column top-8 partials --------
            # t-grouped so tile-t work starts as soon as chunk t lands.
            # Max8 partials read straight from the transpose PSUM (no raw
            # SBUF copy); per-block combine of the 3x8 partials follows.
            cpart = pp.tile([128, CB * 24], F32, tag="cpart")  # per-(j,t) top8
            c8all = pp.tile([128, CB * 8], F32, tag="c8all")
            r8 = pp.tile([128, RT * 8], F32, tag="r8")
            for t in range(RT):
                for j in range(CB):
                    pt = ps1.tile([128, 128], F32, tag="pt")
                    nc.tensor.transpose(
                        pt[:], Xt(t)[:, j * 128:(j + 1) * 128], idn[:])
                    nc.vector.max(
                        out=cpart[:, j * 24 + t * 8:j * 24 + t * 8 + 8],
                        in_=pt[:])
            for j in range(CB):
                nc.vector.max(out=c8all[:, j * 8:j * 8 + 8],
                              in_=cpart[:, j * 24:(j + 1) * 24])
            c3all = pp.tile([128, CB * 3], F32, tag="c3all")
            nc.vector.tensor_copy(
                c3all[:].rearrange("p (j s) -> p j s", s=3),
                c8all[:].rearrange("p (j s) -> p j s", s=8)[:, :, 0:3])
            nc.sync.dma_start(cp_in[:], c3all[:])

            # ---------- collective 1: AllGather column partials ----------
            nc.gpsimd.collective_compute(
                "AllGather", OP.bypass, replica_groups=groups,
                ins=[cp_in[:]], outs=[cp_out[:]])

            gath = pp.tile([128, NCORES * CB * 3], F32, tag="gath")
            nc.sync.dma_start(
                gath[:].rearrange("p (k f) -> p k f", k=NCORES),
                cp_out[:].rearrange("(k p) f -> p k f", p=128))

            # ---------- P2 (during gather): row mask + masked transposes ---
            # row threshold per row is a per-partition scalar in row layout:
            # X_t <- [X_t >= max(r8_t[2], T0)] * X_t  (one fused DVE pass)
            rth = pp.tile([128, RT], F32, tag="rth")
            XT = pp.tile([128, CB * SH], F32, tag="XT")  # [p=col, (j, r)]
            for t in range(RT):
                # row top-8 AFTER the collective path is packed, so the DVE
                # queue drains the gather-critical work first
                nc.vector.max(out=r8[:, t * 8:t * 8 + 8], in_=Xt(t))
                nc.vector.tensor_scalar_max(rth[:, t:t + 1],
                                            r8[:, t * 8 + 2:t * 8 + 3], T0)
                nc.vector.scalar_tensor_tensor(
                    Xt(t), Xt(t), rth[:, t:t + 1], Xt(t),
                    OP.is_ge, OP.mult)
                for j in range(CB):
                    pt2 = psw.tile([128, 128], F32, tag="ptw")
                    nc.tensor.transpose(
                        pt2[:], Xt(t)[:, j * 128:(j + 1) * 128], idn[:])
                    _act_copy(
                        nc,
                        XT[:, j * SH + t * 128: j * SH + (t + 1) * 128],
                        pt2[:])

            # ---------- P3: combine -> exact column thresholds ----------
            cm8 = pp.tile([128, CB * 8], F32, tag="cm8")
            gv = gath[:].rearrange("p (k j s) -> p j k s", k=NCORES, s=3)
            for j in range(CB):
                nc.vector.max(out=cm8[:, j * 8:j * 8 + 8], in_=gv[:, j])

            # ---------- P4+P5: column mask fused with Gram ----------
            # per block j: XTr_j <- [XT_j >= tc_j] * XT_j (tc_j per-partition;
            # compare on exact f32, product rounded to fp32r on write)
            # then PE accumulates Bt[m, r] += m2_j^T @ XTr_j  in fp32r.
            # DVE handles most blocks with a fused scalar_tensor_tensor; the
            # gpsimd (no TensorScalarPtr support) takes the tail blocks with
            # a 2-pass broadcast form.
            XTr = pp.tile([128, CB * SH], F32R, tag="XTr")
            psB = psc.tile([6, SH], F32, tag="psB")
            for j in range(CB):
                nc.vector.scalar_tensor_tensor(
                    XTr[:, j * SH:(j + 1) * SH],
                    XT[:, j * SH:(j + 1) * SH],
                    cm8[:, j * 8 + 2:j * 8 + 3],
                    XT[:, j * SH:(j + 1) * SH],
                    OP.is_ge, OP.mult)
                nc.tensor.matmul(
                    psB[:],
                    m2t_s[:, j * 6:(j + 1) * 6],
                    XTr[:, j * SH:(j + 1) * SH],
                    start=(j == 0), stop=(j == CB - 1))
            Bt = sp.tile([6, SH], F32, tag="Bt")
            _act_copy(nc, Bt[:], psB[:])
            # stage 2: C[a, m] = sum_r M1[r, a] B[r, m]
            Bs = sp.tile([128, RT * 6], F32R, tag="Bs")
            for t in range(RT):
                pb = psa.tile([128, 6], F32, tag="pb")
                nc.tensor.transpose(pb[:], Bt[:, t * 128:(t + 1) * 128],
                                    idn[0:6, 0:6])
                _act_copy(nc, Bs[:, t * 6:(t + 1) * 6], pb[:])
            pc1 = psc.tile([6, 6], F32, tag="pc1")
            for t in range(RT):
                nc.tensor.matmul(pc1[:],
                                 m1t_s[:, t * 6:(t + 1) * 6],
                                 Bs[:, t * 6:(t + 1) * 6],
                                 start=(t == 0), stop=(t == RT - 1))
            Cp = sp.tile([6, 6], F32, tag="Cp")
            _act_copy(nc, Cp[:], pc1[:])
            nc.sync.dma_start(cr_in[:], Cp[:])

            # ---------- collective 2: AllReduce 6x6 Gram ----------
            nc.gpsimd.collective_compute(
                "AllReduce", OP.add, replica_groups=groups,
                ins=[cr_in[:]], outs=[cr_out[:]])

            # ---------- tail ----------
            _tail(nc, pp, sp, ps, cps, idn, cr_out, mshuf, out_d)

    nc.compile()
    return nc


def _transpose(nc, ps, sp, in_sb, n, idn, tag):
    """PE-transpose square [n, n] SBUF -> new SBUF tile."""
    pt = ps.tile([n, n], F32, tag="tps")
    nc.tensor.transpose(pt[:], in_sb, idn[:n, :n])
    ot = sp.tile([n, n], F32, tag=f"ot_{tag}")
    _dcp(nc, ot[:], pt[:])
    return ot


def _pow50(nc, ps, sp, m_sb, n, tag):
    """Direction of M^50 v via rescaled squarings M <- 2*(M@M);
    M50 = 2*((2*(M32@M16)) @ M2). All operands symmetric."""
    powers = {}
    cur = m_sb
    for i in range(1, 6):  # M2, M4, M8, M16, M32
        pm = ps.tile([n, n], F32, tag="tps")
        nc.tensor.matmul(pm[:], cur, cur, start=True, stop=True)
        nxt = sp.tile([n, n], F32, tag=f"pws_{tag}_{i}")
        _dcp(nc, nxt[:], pm[:], scale=2.0)
        powers[2 ** i] = nxt
        cur = nxt[:]
    pm = ps.tile([n, n], F32, tag="tps")
    nc.tensor.matmul(pm[:], powers[32][:], powers[16][:], start=True, stop=True)
    m48 = sp.tile([n, n], F32, tag=f"pws_{tag}_48")
    _dcp(nc, m48[:], pm[:], scale=2.0)
    pm = ps.tile([n, n], F32, tag="tps")
    nc.tensor.matmul(pm[:], m48[:], powers[2][:], start=True, stop=True)
    m50 = sp.tile([n, n], F32, tag=f"pws_{tag}_50")
    _dcp(nc, m50[:], pm[:], scale=2.0)
    return m50


def _tail(nc, pp, sp, ps, cps, idn, cr_out, mshuf, out_d):
    """Hartley scalars, L-transform to C2, Mmat, power chains, projection."""
    i9h = cps[0:9, C_I9H:C_I9H + 9]
    et69 = cps[0:6, C_ET69:C_ET69 + 9]
    i3c = cps[0:3, C_I3:C_I3 + 3]
    v09 = cps[0:9, C_V09:C_V09 + 1]
    v06 = cps[0:6, C_V06:C_V06 + 1]
    sel1 = cps[0:3, C_SEL1:C_SEL1 + 6]
    sel2 = cps[0:3, C_SEL2:C_SEL2 + 6]
    e5 = cps[0:6, C_E5:C_E5 + 1]

    def e6row(j):  # I6 row j as [1, 6] on partition 0
        return cps[0:1, C_E6F + 6 * j:C_E6F + 6 * j + 6]

    def e3row(k):  # I3 row k as [1, 3] on partition 0
        return cps[0:1, C_E3F + 3 * k:C_E3F + 3 * k + 3]

    def e2row(k):  # I2 row k as [1, 2] on partition 0
        return cps[0:1, C_E2F + 2 * k:C_E2F + 2 * k + 2]

    Cr = sp.tile([6, 6], F32, tag="Cr")
    nc.sync.dma_start(Cr[:], cr_out[:])
    CrT = _transpose(nc, ps, sp, Cr[:], 6, idn, "crt")

    sc = pp.tile([128, 224], F32, tag="tailsc")

    def scv(a, b):
        return sc[0:1, a:b]

    mo_ps = ps.tile([1, 6], F32, tag="tps")
    nc.tensor.matmul(mo_ps[:], e5, CrT[:], start=True, stop=True)
    _dcp(nc, scv(0, 6), mo_ps[:])              # side1 moments (tilde)
    mo_ps2 = ps.tile([1, 6], F32, tag="tps")
    nc.tensor.matmul(mo_ps2[:], e5, Cr[:], start=True, stop=True)
    _dcp(nc, scv(6, 12), mo_ps2[:])            # side2 moments (tilde)

    def pair(k):  # element k of each side: free idxs (k, k+6)
        return sc[0:1, 0:12].rearrange("p (g d) -> p d g", g=2)[:, k, :]

    Sxx, Sx, Syy, Sy, Sw = pair(0), pair(2), pair(3), pair(4), pair(5)
    ws = scv(12, 14); nc.vector.tensor_scalar_add(ws, Sw, EPS)
    rws = scv(14, 16); nc.vector.reciprocal(rws, ws)
    cx = scv(16, 18); nc.vector.tensor_tensor(cx, Sx, rws, OP.mult)
    cy = scv(18, 20); nc.vector.tensor_tensor(cy, Sy, rws, OP.mult)
    t_a = scv(20, 22); nc.vector.tensor_tensor(t_a, cx, Sx, OP.mult)
    t_b = scv(22, 24); nc.vector.tensor_tensor(t_b, cy, Sy, OP.mult)
    cdS = scv(24, 26); nc.vector.tensor_tensor(cdS, t_a, t_b, OP.add)
    u_a = scv(26, 28); nc.vector.tensor_tensor(u_a, cx, cx, OP.mult)
    u_b = scv(28, 30); nc.vector.tensor_tensor(u_b, cy, cy, OP.mult)
    c2_ = scv(30, 32); nc.vector.tensor_tensor(c2_, u_a, u_b, OP.add)
    sq_ = scv(32, 34); nc.vector.tensor_tensor(sq_, Sxx, Syy, OP.add)
    n2c = scv(34, 36); nc.vector.tensor_scalar_mul(n2c, cdS, -2.0)
    c2w = scv(36, 38); nc.vector.tensor_tensor(c2w, c2_, Sw, OP.mult)
    m_ = scv(38, 40); nc.vector.tensor_tensor(m_, sq_, n2c, OP.add)
    m2_ = scv(40, 42); nc.vector.tensor_tensor(m2_, m_, c2w, OP.add)
    md2 = scv(42, 44); nc.vector.tensor_tensor(md2, m2_, rws, OP.mult)
    md2e = scv(44, 46); nc.vector.tensor_scalar_add(md2e, md2, EPS)
    md = scv(46, 48); nc.scalar.activation(md, md2e, AF.Sqrt)
    mde = scv(48, 50); nc.vector.tensor_scalar_add(mde, md, EPS)
    rmd = scv(50, 52); nc.vector.reciprocal(rmd, mde)
    s_ = scv(52, 54); nc.vector.tensor_scalar_mul(s_, rmd, SQRT2)

    # L-matrix ingredients (tilde-coord scalars)
    ss = scv(54, 56); nc.vector.tensor_tensor(ss, s_, s_, OP.mult)
    sscx = scv(56, 58); nc.vector.tensor_tensor(sscx, ss, cx, OP.mult)
    sscy = scv(58, 60); nc.vector.tensor_tensor(sscy, ss, cy, OP.mult)
    n2sscx = scv(62, 64); nc.vector.tensor_scalar_mul(n2sscx, sscx, -2.0)
    nsscy = scv(64, 66); nc.vector.tensor_scalar_mul(nsscy, sscy, -1.0)
    nsscx = scv(66, 68); nc.vector.tensor_scalar_mul(nsscx, sscx, -1.0)
    n2sscy = scv(68, 70); nc.vector.tensor_scalar_mul(n2sscy, sscy, -2.0)
    scx = scv(70, 72); nc.vector.tensor_tensor(scx, s_, cx, OP.mult)
    scy = scv(72, 74); nc.vector.tensor_tensor(scy, s_, cy, OP.mult)
    nscx = scv(74, 76); nc.vector.tensor_scalar_mul(nscx, scx, -1.0)
    nscy = scv(76, 78); nc.vector.tensor_scalar_mul(nscy, scy, -1.0)
    sscxcx = scv(78, 80); nc.vector.tensor_tensor(sscxcx, sscx, cx, OP.mult)
    sscxcy = scv(80, 82); nc.vector.tensor_tensor(sscxcy, sscx, cy, OP.mult)
    sscycy = scv(82, 84); nc.vector.tensor_tensor(sscycy, sscy, cy, OP.mult)

    # raw-coord Hartley scalars for the final T1/T2 (x = c0 + x~/s0):
    # s_raw = s0*s~ ; s_raw*cx_raw = s~*(cx~ + s0*c0)
    sr = scv(84, 86); nc.vector.tensor_scalar_mul(sr, s_, S0)
    cxr = scv(86, 88); nc.vector.tensor_scalar_add(cxr, cx, S0 * C0X)
    cyr = scv(88, 90); nc.vector.tensor_scalar_add(cyr, cy, S0 * C0Y)
    u1_ = scv(90, 92); nc.vector.tensor_tensor(u1_, s_, cxr, OP.mult)
    u2_ = scv(92, 94); nc.vector.tensor_tensor(u2_, s_, cyr, OP.mult)
    nscxr = scv(94, 96); nc.vector.tensor_scalar_mul(nscxr, u1_, -1.0)
    nscyr = scv(96, 98); nc.vector.tensor_scalar_mul(nscyr, u2_, -1.0)

    # L^T row vectors for rank-1 assembly: side s base 100+36s, row j at +6j.
    lrows = sc[0:1, 100:172]
    nc.vector.memset(lrows, 0.0)
    lv = lrows.rearrange("p (s k) -> p k s", s=2)  # [1, 36, 2]

    def lwrite(k, src):
        nc.vector.tensor_copy(lv[:, k, :], src)

    lwrite(0, ss)        # row0: [ss, 0, n2sscx, 0, 0, sscxcx]
    lwrite(2, n2sscx)
    lwrite(5, sscxcx)
    lwrite(7, ss)        # row1: [0, ss, nsscy, 0, nsscx, sscxcy]
    lwrite(8, nsscy)
    lwrite(10, nsscx)
    lwrite(11, sscxcy)
    lwrite(14, s_)       # row2: [0, 0, s, 0, 0, nscx]
    lwrite(17, nscx)
    lwrite(21, ss)       # row3: [0, 0, 0, ss, n2sscy, sscycy]
    lwrite(22, n2sscy)
    lwrite(23, sscycy)
    lwrite(28, s_)       # row4: [0, 0, 0, 0, s, nscy]
    lwrite(29, nscy)
    nc.vector.memset(lv[:, 35, :], 1.0)   # row5 = e5

    def lrow(side, j):
        return sc[0:1, 100 + 36 * side + 6 * j:100 + 36 * side + 6 * j + 6]

    # L1T/L2T via rank-1 accumulation: column j of L^T = row j of L
    def build_LT(side, tag):
        lps = ps.tile([6, 6], F32, tag="tps")
        for j in range(6):
            nc.tensor.matmul(lps[:], lrow(side, j), e6row(j),
                             start=(j == 0), stop=(j == 5))
        lt = sp.tile([6, 6], F32, tag=tag)
        _dcp(nc, lt[:], lps[:])
        return lt

    L1Ts = build_LT(0, "L1Ts")
    L2Ts = build_LT(1, "L2Ts")

    # C2^T = L2 @ (L1 @ C)^T
    zps = ps.tile([6, 6], F32, tag="tps")
    nc.tensor.matmul(zps[:], L1Ts[:], Cr[:], start=True, stop=True)   # L1 @ C
    Zs = sp.tile([6, 6], F32, tag="Zs")
    _dcp(nc, Zs[:], zps[:])
    ZTs = _transpose(nc, ps, sp, Zs[:], 6, idn, "zt")
    c2ps = ps.tile([6, 6], F32, tag="tps")
    nc.tensor.matmul(c2ps[:], L2Ts[:], ZTs[:], start=True, stop=True)  # C2^T
    C2Ts = sp.tile([6, 6], F32, tag="C2Ts")
    _dcp(nc, C2Ts[:], c2ps[:])

    # G2 = E C2 E^T : G2[3a+b, 3c+d] = C2[pair(a,b), pair(c,d)]
    z2ps = ps.tile([6, 9], F32, tag="tps")
    nc.tensor.matmul(z2ps[:], C2Ts[:], et69, start=True, stop=True)  # C2 E^T
    Z2s = sp.tile([6, 9], F32, tag="Z2s")
    _dcp(nc, Z2s[:], z2ps[:])
    g_ps = ps.tile([9, 9], F32, tag="tps")
    nc.tensor.matmul(g_ps[:], et69, Z2s[:], start=True, stop=True)    # E @ Z
    G2 = sp.tile([9, 9], F32, tag="G2")
    _dcp(nc, G2[:], g_ps[:])

    # Mmat[3p+q, 3r+s] = G2[3p+r, 3q+s]: bounce via DRAM, 3 row reads
    nc.sync.dma_start(mshuf[:], G2[:])
    Mmat = sp.tile([9, 9], F32, tag="Mmat")
    for p in range(3):
        # Mmat[3p+q, 3r+s] <- mshuf[27p + 9r + 3q + s]; dims (q, r, s)
        nc.sync.dma_start(
            Mmat[3 * p:3 * p + 3, :].rearrange("q (r s) -> q r s", s=3),
            mshuf[:].rearrange("(p q1 r s) -> p q1 r s", p=3, q1=3, r=3)
            .transpose([0, 2, 1, 3])[p])

    # shifted scaled 9x9: Msp = Mmat/(2 lam) - I/2 (sign irrelevant, even pow)
    dg = sp.tile([9, 9], F32, tag="dg")
    nc.vector.tensor_tensor(dg[:], Mmat[:], i9h, OP.mult)  # diag/2
    lam2 = sp.tile([9, 1], F32, tag="lam2")
    nc.vector.tensor_reduce(lam2[:], dg[:], AX.X, OP.add)
    lam2r = sp.tile([9, 1], F32, tag="lam2r")
    nc.gpsimd.partition_all_reduce(lam2r[:], lam2[:], channels=9,
                                   reduce_op=bass_isa.ReduceOp.add)
    lam4 = sp.tile([9, 1], F32, tag="lam4")
    nc.vector.tensor_scalar_mul(lam4[:], lam2r[:], 4.0)  # = 2*lam
    inv2l = sp.tile([9, 1], F32, tag="inv2l")
    nc.vector.reciprocal(inv2l[:], lam4[:])
    Msp = sp.tile([9, 9], F32, tag="Msp")
    nc.vector.scalar_tensor_tensor(Msp[:], Mmat[:], inv2l[:], i9h,
                                   OP.mult, OP.subtract)
    M50 = _pow50(nc, ps, sp, Msp[:], 9, "m9")

    w9ps = ps.tile([1, 9], F32, tag="tps")
    nc.tensor.matmul(w9ps[:], v09, M50[:], start=True, stop=True)
    w9 = sp.tile([1, 9], F32, tag="w9")
    _dcp(nc, w9[:], w9ps[:])
    w9sq = sp.tile([1, 9], F32, tag="w9sq")
    nc.vector.tensor_tensor(w9sq[:], w9[:], w9[:], OP.mult)
    nn9 = sp.tile([1, 1], F32, tag="nn9")
    nc.vector.tensor_reduce(nn9[:], w9sq[:], AX.X, OP.add)
    sr9 = sp.tile([1, 1], F32, tag="sr9")
    nc.scalar.activation(sr9[:], nn9[:], AF.Sqrt)
    rs9 = sp.tile([1, 1], F32, tag="rs9")
    nc.vector.reciprocal(rs9[:], sr9[:])
    v9 = sp.tile([1, 9], F32, tag="v9")
    nc.vector.tensor_tensor(v9[:], w9[:], rs9[:].to_broadcast([1, 9]), OP.mult)

    # Eraw [3,3]: row k = v9[3k:3k+3], via rank-1 matmuls
    erps = ps.tile([3, 3], F32, tag="tps")
    for k in range(3):
        nc.tensor.matmul(erps[:], e3row(k), v9[0:1, 3 * k:3 * k + 3],
                         start=(k == 0), stop=(k == 2))
    Eraw = sp.tile([3, 3], F32, tag="Eraw")
    _dcp(nc, Eraw[:], erps[:])

    # T1m/T2m [3,3] from raw Hartley scalars via rank-1 matmuls.
    # per side 16 slots at 176+16s: buf6 = [sr,0,0,0,sr,0] at +0,
    # col2 = [nscxr, nscyr, 1] at +8.
    tcols = sc[0:1, 176:208]
    nc.vector.memset(tcols, 0.0)
    tcv = tcols.rearrange("p (s k) -> p k s", s=2)  # [1, 16, 2]
    nc.vector.tensor_copy(tcv[:, 0, :], sr)
    nc.vector.tensor_copy(tcv[:, 4, :], sr)
    nc.vector.tensor_copy(tcv[:, 8, :], nscxr)
    nc.vector.tensor_copy(tcv[:, 9, :], nscyr)
    nc.vector.memset(tcv[:, 10, :], 1.0)

    def tcol(side, off, ln):
        return sc[0:1, 176 + 16 * side + off:176 + 16 * side + off + ln]

    def build_T(side, tag):
        tps_ = ps.tile([3, 3], F32, tag="tps")
        nc.tensor.matmul(tps_[:], tcol(side, 0, 3), e3row(0),
                         start=True, stop=False)
        nc.tensor.matmul(tps_[:], tcol(side, 3, 3), e3row(1),
                         start=False, stop=False)
        nc.tensor.matmul(tps_[:], tcol(side, 8, 3), e3row(2),
                         start=False, stop=True)
        tm = sp.tile([3, 3], F32, tag=tag)
        _dcp(nc, tm[:], tps_[:])
        return tm

    T1m = build_T(0, "T1m")
    T2m = build_T(1, "T2m")

    # E = T2^T E_raw T1 (and E^T)
    a1ps = ps.tile([3, 3], F32, tag="tps")
    nc.tensor.matmul(a1ps[:], T2m[:], Eraw[:], start=True, stop=True)
    A1 = sp.tile([3, 3], F32, tag="A1")
    _dcp(nc, A1[:], a1ps[:])
    A1T = _transpose(nc, ps, sp, A1[:], 3, idn, "a1t")
    etps = ps.tile([3, 3], F32, tag="tps")
    nc.tensor.matmul(etps[:], T1m[:], A1T[:], start=True, stop=True)
    ETs = sp.tile([3, 3], F32, tag="ETs")
    _dcp(nc, ETs[:], etps[:])
    Es = _transpose(nc, ps, sp, ETs[:], 3, idn, "es")

    # B = E^T E ; blockdiag 6x6 chain for v1 (max) and v3 (min)
    bps = ps.tile([3, 3], F32, tag="tps")
    nc.tensor.matmul(bps[:], Es[:], Es[:], start=True, stop=True)
    Bm = sp.tile([3, 3], F32, tag="Bm")
    _dcp(nc, Bm[:], bps[:])
    dg3 = sp.tile([3, 3], F32, tag="dg3")
    nc.vector.tensor_tensor(dg3[:], Bm[:], i3c, OP.mult)
    lb = sp.tile([3, 1], F32, tag="lb")
    nc.vector.tensor_reduce(lb[:], dg3[:], AX.X, OP.add)
    lbr = sp.tile([3, 1], F32, tag="lbr")
    nc.gpsimd.partition_all_reduce(lbr[:], lb[:], channels=3,
                                   reduce_op=bass_isa.ReduceOp.add)
    invlb = sp.tile([3, 1], F32, tag="invlb")
    nc.vector.reciprocal(invlb[:], lbr[:])
    Bs3 = sp.tile([3, 3], F32, tag="Bs3")
    nc.vector.tensor_scalar_mul(Bs3[:], Bm[:], invlb[:])
    IB = sp.tile([3, 3], F32, tag="IB")
    nc.vector.tensor_tensor(IB[:], i3c, Bs3[:], OP.subtract)
    bdps = ps.tile([6, 6], F32, tag="tps")
    nc.tensor.matmul(bdps[:, 0:3], sel1, Bs3[:], start=True, stop=True)
    nc.tensor.matmul(bdps[:, 3:6], sel2, IB[:], start=True, stop=True)
    BD = sp.tile([6, 6], F32, tag="BD")
    _dcp(nc, BD[:], bdps[:])
    BD50 = _pow50(nc, ps, sp, BD[:], 6, "m6")

    w6ps = ps.tile([1, 6], F32, tag="tps")
    nc.tensor.matmul(w6ps[:], v06, BD50[:], start=True, stop=True)
    w6 = sp.tile([1, 6], F32, tag="w6")
    _dcp(nc, w6[:], w6ps[:])
    w6sq = sp.tile([1, 6], F32, tag="w6sq")
    nc.vector.tensor_tensor(w6sq[:], w6[:], w6[:], OP.mult)
    nn6 = sp.tile([1, 2], F32, tag="nn6")
    nc.vector.tensor_reduce(nn6[:].unsqueeze(2),
                            w6sq[:].rearrange("p (g d) -> p g d", g=2), AX.X,
                            OP.add)
    sr6 = sp.tile([1, 2], F32, tag="sr6")
    nc.scalar.activation(sr6[:], nn6[:], AF.Sqrt)
    rs6 = sp.tile([1, 2], F32, tag="rs6")
    nc.vector.reciprocal(rs6[:], sr6[:])
    vv = sp.tile([1, 6], F32, tag="vv")
    nc.vector.tensor_tensor(
        vv[:].rearrange("p (g d) -> p g d", g=2),
        w6[:].rearrange("p (g d) -> p g d", g=2),
        rs6[:].unsqueeze(2).to_broadcast([1, 2, 3]), OP.mult)

    # v2 = cross(v3, v1), normalized with EPS (as reference)
    aa = sp.tile([1, 6], F32, tag="aa")
    nc.vector.tensor_copy(
        aa[:].rearrange("p (r d) -> p r d", r=2),
        vv[:, 3:6].unsqueeze(1).to_broadcast([1, 2, 3]))
    bb = sp.tile([1, 6], F32, tag="bb")
    nc.vector.tensor_copy(
        bb[:].rearrange("p (r d) -> p r d", r=2),
        vv[:, 0:3].unsqueeze(1).to_broadcast([1, 2, 3]))
    cr1 = sp.tile([1, 3], F32, tag="cr1")
    nc.vector.tensor_tensor(cr1[:], aa[:, 1:4], bb[:, 2:5], OP.mult)
    cr2 = sp.tile([1, 3], F32, tag="cr2")
    nc.vector.tensor_tensor(cr2[:], aa[:, 2:5], bb[:, 1:4], OP.mult)
    v2r = sp.tile([1, 3], F32, tag="v2r")
    nc.vector.tensor_tensor(v2r[:], cr1[:], cr2[:], OP.subtract)
    v2sq = sp.tile([1, 3], F32, tag="v2sq")
    nc.vector.tensor_tensor(v2sq[:], v2r[:], v2r[:], OP.mult)
    nn2 = sp.tile([1, 1], F32, tag="nn2")
    nc.vector.tensor_reduce(nn2[:], v2sq[:], AX.X, OP.add)
    sr2 = sp.tile([1, 1], F32, tag="sr2")
    nc.scalar.activation(sr2[:], nn2[:], AF.Sqrt)
    sr2e = sp.tile([1, 1], F32, tag="sr2e")
    nc.vector.tensor_scalar_add(sr2e[:], sr2[:], EPS)
    rs2 = sp.tile([1, 1], F32, tag="rs2")
    nc.vector.reciprocal(rs2[:], sr2e[:])
    v2 = sp.tile([1, 3], F32, tag="v2")
    nc.vector.tensor_tensor(v2[:], v2r[:], rs2[:].to_broadcast([1, 3]), OP.mult)

    # Vr [2,3] (rows v1, v2) and Vc [3,2] (cols v1, v2) via rank-1 matmuls
    vrps = ps.tile([2, 3], F32, tag="tps")
    nc.tensor.matmul(vrps[:], e2row(0), vv[:, 0:3], start=True, stop=False)
    nc.tensor.matmul(vrps[:], e2row(1), v2[:], start=False, stop=True)
    Vr = sp.tile([2, 3], F32, tag="Vr")
    _dcp(nc, Vr[:], vrps[:])
    vcps = ps.tile([3, 2], F32, tag="tps")
    nc.tensor.matmul(vcps[:], vv[:, 0:3], e2row(0), start=True, stop=False)
    nc.tensor.matmul(vcps[:], v2[:], e2row(1), start=False, stop=True)
    Vc = sp.tile([3, 2], F32, tag="Vc")
    _dcp(nc, Vc[:], vcps[:])

    evps = ps.tile([2, 3], F32, tag="tps")
    nc.tensor.matmul(evps[:], Vc[:], ETs[:], start=True, stop=True)
    Evr = sp.tile([2, 3], F32, tag="Evr")
    _dcp(nc, Evr[:], evps[:])
    evsq = sp.tile([2, 3], F32, tag="evsq")
    nc.vector.tensor_tensor(evsq[:], Evr[:], Evr[:], OP.mult)
    ss2 = sp.tile([2, 1], F32, tag="ss2")
    nc.vector.tensor_reduce(ss2[:], evsq[:], AX.X, OP.add)
    sv = sp.tile([2, 1], F32, tag="sv")
    nc.scalar.activation(sv[:], ss2[:], AF.Sqrt)
    ssum = sp.tile([2, 1], F32, tag="ssum")
    nc.gpsimd.partition_all_reduce(ssum[:], sv[:], channels=2,
                                   reduce_op=bass_isa.ReduceOp.add)
    savg = sp.tile([2, 1], F32, tag="savg")
    nc.vector.tensor_scalar_mul(savg[:], ssum[:], 0.5)
    sve = sp.tile([2, 1], F32, tag="sve")
    nc.vector.tensor_scalar_add(sve[:], sv[:], EPS)
    rsv = sp.tile([2, 1], F32, tag="rsv")
    nc.vector.reciprocal(rsv[:], sve[:])
    f2 = sp.tile([2, 1], F32, tag="f2")
    nc.vector.tensor_tensor(f2[:], rsv[:], savg[:], OP.mult)
    U2 = sp.tile([2, 3], F32, tag="U2")
    nc.vector.tensor_scalar_mul(U2[:], Evr[:], f2[:])
    ops_ = ps.tile([3, 3], F32, tag="tps")
    nc.tensor.matmul(ops_[:], U2[:], Vr[:], start=True, stop=True)
    outs = sp.tile([3, 3], F32, tag="outs")
    _dcp(nc, outs[:], ops_[:])
    nc.sync.dma_start(out_d[:], outs[:])


def make_in_maps(P, K):
    """Host-side shard + constant prep: list of 8 input dicts."""
    P = np.asarray(P, np.float32)
    K = np.asarray(K, np.float32)
    Pc = np.ascontiguousarray(P[:N, :N])
    M, cpack = host_constants(K)
    m2t = _tile128(M, CB)
    ident = np.eye(128, dtype=np.float32)
    in_maps = []
    for k in range(NCORES):
        sh = Pc[k * SH:(k + 1) * SH]
        in_maps.append({
            "xin": _tile128(sh, RT),
            "m1s": _tile128(M[k * SH:(k + 1) * SH], RT),
            "m2t": m2t,
            "ident": ident,
            "cpack": cpack,
        })
    return in_maps


_NC_CACHE = {}


def kernel(P, K):
    from concourse.bass_utils import run_bass_kernel_spmd
    if "nc" not in _NC_CACHE:
        _NC_CACHE["nc"] = build_nc()
    nc = _NC_CACHE["nc"]
    in_maps = make_in_maps(P, K)
    res = run_bass_kernel_spmd(nc, in_maps, core_ids=list(range(NCORES)))
    return np.asarray(res.results[0]["out"], np.float32)


# revision 39
# speedup vs baseline: 1.0218x; 1.0218x over previous
"""Trainium2 Bass kernel for nn_EssentialMatrixEstimator.

Distribution: data-parallel over the N=3072 rows of Pc across 8 cores
(384 rows each).

Math: the (N*M, 9) epipolar design-matrix Gram collapses to a single 6x6
monomial Gram C = M1^T W M2 computed in HOST-pre-centered coordinates
x~ = s0*(x - c0) (no cancellation).  The Hartley normalization is a 6x6
linear map on monomials, so the normalized Gram is C2 = L1 C L2^T with
L1/L2 built on-device from the Hartley scalars (which live in row/col 5
of C).  Mmat (9x9) is then a pure index expansion of C2.  One AllGather
(column top-3 partials) + one AllReduce (6x6 Gram) total.
W is the bidirectional-top3 (+ >0.01) masked score matrix; exact top-3
via Max8 (column partials read straight from the transpose PSUM, so the
AllGather posts as soon as the input DMA drains).  Row masking is a
single fused scalar_tensor_tensor pass in row-layout during the gather;
masked data is re-transposed during the same window.  Gram matmuls run
as fp32r with the 6-wide monomial matrix PE-stationary.  The 50-step
power iterations run as rescaled repeated squaring (M <- 2*(M@M)).
Small 3x3/2x3 matrices are assembled from partition-0 scalars via PE
rank-1 (outer-product) matmuls instead of DRAM staging round trips.
"""

import os

os.environ.setdefault("JAX_PLATFORMS", "axon")

import numpy as np

import concourse.bass as bass
import concourse.bass_isa as bass_isa
import concourse.mybir as mybir
import concourse.bacc as bacc
import concourse.tile as tile

NCORES = 8
N = 3072
SH = N // NCORES          # 384 rows per core
RT = SH // 128            # 3 row tiles per core
CB = N // 128             # 24 column blocks
F32 = mybir.dt.float32
F32R = mybir.dt.float32r
AF = mybir.ActivationFunctionType
OP = mybir.AluOpType
AX = mybir.AxisListType

EPS = 1e-8
SQRT2 = 1.4142135623730951
INV_SQRT3 = 1.0 / 1.7320508075688772
T0 = float(np.nextafter(np.float32(0.01), np.float32(1)))  # x > 0.01 == x >= T0
H, W = 64, 64

# host pre-centering of the camera-plane grid coordinates
S0 = 20.0
C0X = -0.577
C0Y = -0.432

# colmask runs entirely on DVE: the Pool/gpsimd engine supports neither
# TensorScalarPtr nor broadcast (stride-0) TensorTensor operands
DVE_CM = CB

# cpack const layout (tensor [9, 48]): column ranges
C_I9H = 0      # I9 * 0.5          [9, 9]
C_ET69 = 9     # E^T selector      [6, 9]
C_I3 = 18      # I3                [3, 3]
C_V09 = 21     # full(1/3)         [9, 1]
C_V06 = 22     # full(1/sqrt3)     [6, 1]
C_SEL1 = 23    # [I3 | 0]          [3, 6]
C_SEL2 = 29    # [0 | I3]          [3, 6]
C_E5 = 35      # e5 selector       [6, 1]
C_I6 = 36      # I6                [6, 6]
C_E6F = 48     # flat I6 rows      [1, 36] (row j at 6j, partition 0)
C_E3F = 84     # flat I3 rows      [1, 9]
C_E2F = 93     # flat I2 rows      [1, 4]

PAIRS = [(0, 0), (0, 1), (0, 2), (1, 1), (1, 2), (2, 2)]


def _pidx():
    d = {}
    for i, (a, b) in enumerate(PAIRS):
        d[(a, b)] = i
        d[(b, a)] = i
    return d


def host_constants(K):
    """Pre-centered monomial matrix + packed tail constants (all f32)."""
    idx = np.arange(H * W, dtype=np.float32)
    pix = np.stack([idx % np.float32(W), np.floor(idx / np.float32(W))], -1)
    K_inv = np.linalg.inv(np.asarray(K, np.float32)).astype(np.float32)
    p1h = np.concatenate([pix[:N], np.ones((N, 1), np.float32)], -1)
    pts = (p1h @ K_inv.T)[:, :2].astype(np.float32)  # same grid both sides
    x = (np.float32(S0) * (pts[:, 0] - np.float32(C0X))).astype(np.float32)
    y = (np.float32(S0) * (pts[:, 1] - np.float32(C0Y))).astype(np.float32)
    M = np.stack([x * x, x * y, x, y * y, y, np.ones_like(x)], -1).astype(np.float32)

    cpack = np.zeros((9, 100), np.float32)
    cpack[:9, C_I9H:C_I9H + 9] = 0.5 * np.eye(9, dtype=np.float32)
    pid = _pidx()
    for a in range(3):
        for b in range(3):
            cpack[pid[(a, b)], C_ET69 + 3 * a + b] = 1.0  # ET69[m, 3a+b]
    cpack[:3, C_I3:C_I3 + 3] = np.eye(3, dtype=np.float32)
    cpack[:9, C_V09] = 1.0 / 3.0
    cpack[:6, C_V06] = INV_SQRT3
    cpack[:3, C_SEL1:C_SEL1 + 3] = np.eye(3, dtype=np.float32)
    cpack[:3, C_SEL2 + 3:C_SEL2 + 6] = np.eye(3, dtype=np.float32)
    cpack[5, C_E5] = 1.0
    cpack[:6, C_I6:C_I6 + 6] = np.eye(6, dtype=np.float32)
    cpack[0, C_E6F:C_E6F + 36] = np.eye(6, dtype=np.float32).reshape(-1)
    cpack[0, C_E3F:C_E3F + 9] = np.eye(3, dtype=np.float32).reshape(-1)
    cpack[0, C_E2F:C_E2F + 4] = np.eye(2, dtype=np.float32).reshape(-1)
    return M, cpack


def _tile128(a, ntiles):
    """[ntiles*128, F] -> [128, ntiles*F] with [p, t*F+f] = a[t*128+p, f]."""
    F = a.shape[1]
    return np.ascontiguousarray(
        a.reshape(ntiles, 128, F).transpose(1, 0, 2).reshape(128, ntiles * F)
    )


def _act_copy(nc, out, in_, scale=1.0):
    nc.scalar.activation(out, in_, AF.Copy, scale=scale)


def _dcp(nc, out, in_, scale=None):
    """Tail copies run on DVE (idle there, lower latency than ACT)."""
    if scale is None:
        nc.vector.tensor_copy(out, in_)
    else:
        nc.vector.tensor_scalar_mul(out, in_, scale)


def build_nc():
    """Build the SPMD 8-core Bass program; returns compiled nc."""
    nc = bacc.Bacc("TRN2", target_bir_lowering=False, debug=False,
                   num_devices=NCORES)

    xin = nc.dram_tensor("xin", [128, RT * N], F32, kind="ExternalInput")
    m1s = nc.dram_tensor("m1s", [128, RT * 6], F32R, kind="ExternalInput")
    m2t = nc.dram_tensor("m2t", [128, CB * 6], F32R, kind="ExternalInput")
    ident = nc.dram_tensor("ident", [128, 128], F32, kind="ExternalInput")
    cpk = nc.dram_tensor("cpack", [9, 100], F32, kind="ExternalInput")
    out_d = nc.dram_tensor("out", [3, 3], F32, kind="ExternalOutput")

    cp_in = nc.dram_tensor("cp_in", [128, CB * 3], F32)
    cp_out = nc.dram_tensor("cp_out", [NCORES * 128, CB * 3], F32,
                            addr_space="Shared")
    cr_in = nc.dram_tensor("cr_in", [6, 6], F32)
    cr_out = nc.dram_tensor("cr_out", [6, 6], F32, addr_space="Shared")
    mshuf = nc.dram_tensor("mshuf", [81], F32)

    groups = [list(range(NCORES))]

    with tile.TileContext(nc) as tc:
        with (
            tc.tile_pool(name="persist", bufs=1) as pp,
            tc.tile_pool(name="scratch", bufs=2) as sp,
            tc.tile_pool(name="ps_pt", bufs=3, space="PSUM") as ps1,
            tc.tile_pool(name="ps_w2", bufs=2, space="PSUM") as psw,
            tc.tile_pool(name="ps_tl", bufs=1, space="PSUM") as ps,
            tc.tile_pool(name="ps_acc", bufs=1, space="PSUM") as psa,
            tc.tile_pool(name="ps_c", bufs=1, space="PSUM") as psc,
        ):
            # ---------- P0: loads ----------
            # idn FIRST: every transpose depends on it, and DMA queues drain
            # in issue order.  One dma + one tile per X row tile, so
            # consumers of tile t wait only chunk t.
            idn = pp.tile([128, 128], F32, tag="idn")
            nc.sync.dma_start(idn[:], ident[:])
            cps = pp.tile([9, 100], F32, tag="cpk")
            nc.sync.dma_start(cps[:], cpk[:])
            Xs = []
            for t in range(RT):
                Xi = pp.tile([128, N], F32, tag=f"X{t}")
                nc.sync.dma_start(Xi[:], xin[:, t * N:(t + 1) * N])
                Xs.append(Xi)
            m1t_s = pp.tile([128, RT * 6], F32R, tag="m1")
            nc.sync.dma_start(m1t_s[:], m1s[:])
            m2t_s = pp.tile([128, CB * 6], F32R, tag="m2")
            nc.sync.dma_start(m2t_s[:], m2t[:])

            def Xt(t):
                return Xs[t][:]

            # ---------- P1: raw transposes -> column top-8 partials --------
            # t-grouped so tile-t work starts as soon as chunk t lands.
            # Even blocks: DVE Max8 partials straight from the transpose
            # PSUM.  Odd blocks: Scalar copies the raw transpose to SBUF and
            # ONE DVE Max8 [128, 384] per block runs later — splitting the
            # consumer load across engines shortens the DVE chain that gates
            # the AllGather.  Emitted under high_priority so the scheduler
            # drains this path first.
            cpart = pp.tile([128, CB * 24], F32, tag="cpart")  # per-(j,t) top8
            c8all = pp.tile([128, CB * 8], F32, tag="c8all")
            xtraw = pp.tile([128, (CB // 2) * SH], F32, tag="xtraw")
            r8 = pp.tile([128, RT * 8], F32, tag="r8")
            with tc.high_priority():
                for t in range(RT):
                    for j in range(CB):
                        pt = ps1.tile([128, 128], F32, tag="pt")
                        nc.tensor.transpose(
                            pt[:], Xt(t)[:, j * 128:(j + 1) * 128], idn[:])
                        if j % 2 == 0:
                            nc.vector.max(
                                out=cpart[:, j * 24 + t * 8:j * 24 + t * 8 + 8],
                                in_=pt[:])
                        else:
                            h = j // 2
                            _act_copy(
                                nc,
                                xtraw[:, h * SH + t * 128:h * SH + (t + 1) * 128],
                                pt[:])
                for j in range(0, CB, 2):
                    nc.vector.max(out=c8all[:, j * 8:j * 8 + 8],
                                  in_=cpart[:, j * 24:(j + 1) * 24])
                for j in range(1, CB, 2):
                    h = j // 2
                    nc.vector.max(out=c8all[:, j * 8:j * 8 + 8],
                                  in_=xtraw[:, h * SH:(h + 1) * SH])
                c3all = pp.tile([128, CB * 3], F32, tag="c3all")
                nc.vector.tensor_copy(
                    c3all[:].rearrange("p (j s) -> p j s", s=3),
                    c8all[:].rearrange("p (j s) -> p j s", s=8)[:, :, 0:3])
                nc.sync.dma_start(cp_in[:], c3all[:])

                # ---------- collective 1: AllGather column partials --------
                nc.gpsimd.collective_compute(
                    "AllGather", OP.bypass, replica_groups=groups,
                    ins=[cp_in[:]], outs=[cp_out[:]])

                gath = pp.tile([128, NCORES * CB * 3], F32, tag="gath")
                nc.sync.dma_start(
                    gath[:].rearrange("p (k f) -> p k f", k=NCORES),
                    cp_out[:].rearrange("(k p) f -> p k f", p=128))

            # ---------- P2 (during gather): row mask + masked transposes ---
            # row threshold per row is a per-partition scalar in row layout:
            # X_t <- [X_t >= max(r8_t[2], T0)] * X_t  (one fused DVE pass)
            rth = pp.tile([128, RT], F32, tag="rth")
            XT = pp.tile([128, CB * SH], F32, tag="XT")  # [p=col, (j, r)]
            # de-prioritized so the scheduler never interleaves these big DVE
            # ops into the gather-critical chain above
            with tc.high_priority(offset=-100000):
                for t in range(RT):
                    nc.vector.max(out=r8[:, t * 8:t * 8 + 8], in_=Xt(t))
                    nc.vector.tensor_scalar_max(rth[:, t:t + 1],
                                                r8[:, t * 8 + 2:t * 8 + 3], T0)
                    nc.vector.scalar_tensor_tensor(
                        Xt(t), Xt(t), rth[:, t:t + 1], Xt(t),
                        OP.is_ge, OP.mult)
                    for j in range(CB):
                        pt2 = psw.tile([128, 128], F32, tag="ptw")
                        nc.tensor.transpose(
                            pt2[:], Xt(t)[:, j * 128:(j + 1) * 128], idn[:])
                        _act_copy(
                            nc,
                            XT[:, j * SH + t * 128: j * SH + (t + 1) * 128],
                            pt2[:])

            # ---------- P3: combine -> exact column thresholds ----------
            cm8 = pp.tile([128, CB * 8], F32, tag="cm8")
            gv = gath[:].rearrange("p (k j s) -> p j k s", k=NCORES, s=3)
            for j in range(CB):
                nc.vector.max(out=cm8[:, j * 8:j * 8 + 8], in_=gv[:, j])

            # ---------- P4+P5: column mask fused with Gram ----------
            # per block j: XTr_j <- [XT_j >= tc_j] * XT_j (tc_j per-partition;
            # compare on exact f32, product rounded to fp32r on write)
            # then PE accumulates Bt[m, r] += m2_j^T @ XTr_j  in fp32r.
            # DVE handles most blocks with a fused scalar_tensor_tensor; the
            # gpsimd (no TensorScalarPtr support) takes the tail blocks with
            # a 2-pass broadcast form.
            XTr = pp.tile([128, CB * SH], F32R, tag="XTr")
            psB = psc.tile([6, SH], F32, tag="psB")
            for j in range(CB):
                nc.vector.scalar_tensor_tensor(
                    XTr[:, j * SH:(j + 1) * SH],
                    XT[:, j * SH:(j + 1) * SH],
                    cm8[:, j * 8 + 2:j * 8 + 3],
                    XT[:, j * SH:(j + 1) * SH],
                    OP.is_ge, OP.mult)
                nc.tensor.matmul(
                    psB[:],
                    m2t_s[:, j * 6:(j + 1) * 6],
                    XTr[:, j * SH:(j + 1) * SH],
                    start=(j == 0), stop=(j == CB - 1))
            Bt = sp.tile([6, SH], F32, tag="Bt")
            _act_copy(nc, Bt[:], psB[:])
            # stage 2: C[a, m] = sum_r M1[r, a] B[r, m]
            Bs = sp.tile([128, RT * 6], F32R, tag="Bs")
            for t in range(RT):
                pb = psa.tile([128, 6], F32, tag="pb")
                nc.tensor.transpose(pb[:], Bt[:, t * 128:(t + 1) * 128],
                                    idn[0:6, 0:6])
                _act_copy(nc, Bs[:, t * 6:(t + 1) * 6], pb[:])
            pc1 = psc.tile([6, 6], F32, tag="psB")  # reuse psB's bank
            for t in range(RT):
                nc.tensor.matmul(pc1[:],
                                 m1t_s[:, t * 6:(t + 1) * 6],
                                 Bs[:, t * 6:(t + 1) * 6],
                                 start=(t == 0), stop=(t == RT - 1))
            Cp = sp.tile([6, 6], F32, tag="Cp")
            _act_copy(nc, Cp[:], pc1[:])
            nc.sync.dma_start(cr_in[:], Cp[:])

            # ---------- collective 2: AllReduce 6x6 Gram ----------
            nc.gpsimd.collective_compute(
                "AllReduce", OP.add, replica_groups=groups,
                ins=[cr_in[:]], outs=[cr_out[:]])

            # ---------- tail ----------
            _tail(nc, pp, sp, ps, cps, idn, cr_out, mshuf, out_d)

    nc.compile()
    return nc


def _transpose(nc, ps, sp, in_sb, n, idn, tag):
    """PE-transpose square [n, n] SBUF -> new SBUF tile."""
    pt = ps.tile([n, n], F32, tag="tps")
    nc.tensor.transpose(pt[:], in_sb, idn[:n, :n])
    ot = sp.tile([n, n], F32, tag=f"ot_{tag}")
    _dcp(nc, ot[:], pt[:])
    return ot


def _pow50(nc, ps, sp, m_sb, n, tag):
    """Direction of M^50 v via rescaled squarings M <- 2*(M@M);
    M50 = 2*((2*(M32@M16)) @ M2). All operands symmetric."""
    powers = {}
    cur = m_sb
    for i in range(1, 6):  # M2, M4, M8, M16, M32
        pm = ps.tile([n, n], F32, tag="tps")
        nc.tensor.matmul(pm[:], cur, cur, start=True, stop=True)
        nxt = sp.tile([n, n], F32, tag=f"pws_{tag}_{i}")
        _dcp(nc, nxt[:], pm[:], scale=2.0)
        powers[2 ** i] = nxt
        cur = nxt[:]
    pm = ps.tile([n, n], F32, tag="tps")
    nc.tensor.matmul(pm[:], powers[32][:], powers[16][:], start=True, stop=True)
    m48 = sp.tile([n, n], F32, tag=f"pws_{tag}_48")
    _dcp(nc, m48[:], pm[:], scale=2.0)
    pm = ps.tile([n, n], F32, tag="tps")
    nc.tensor.matmul(pm[:], m48[:], powers[2][:], start=True, stop=True)
    m50 = sp.tile([n, n], F32, tag=f"pws_{tag}_50")
    _dcp(nc, m50[:], pm[:], scale=2.0)
    return m50


def _tail(nc, pp, sp, ps, cps, idn, cr_out, mshuf, out_d):
    """Hartley scalars, L-transform to C2, Mmat, power chains, projection."""
    i9h = cps[0:9, C_I9H:C_I9H + 9]
    et69 = cps[0:6, C_ET69:C_ET69 + 9]
    i3c = cps[0:3, C_I3:C_I3 + 3]
    v09 = cps[0:9, C_V09:C_V09 + 1]
    v06 = cps[0:6, C_V06:C_V06 + 1]
    sel1 = cps[0:3, C_SEL1:C_SEL1 + 6]
    sel2 = cps[0:3, C_SEL2:C_SEL2 + 6]
    e5 = cps[0:6, C_E5:C_E5 + 1]

    def e6row(j):  # I6 row j as [1, 6] on partition 0
        return cps[0:1, C_E6F + 6 * j:C_E6F + 6 * j + 6]

    def e3row(k):  # I3 row k as [1, 3] on partition 0
        return cps[0:1, C_E3F + 3 * k:C_E3F + 3 * k + 3]

    def e2row(k):  # I2 row k as [1, 2] on partition 0
        return cps[0:1, C_E2F + 2 * k:C_E2F + 2 * k + 2]

    Cr = sp.tile([6, 6], F32, tag="Cr")
    nc.sync.dma_start(Cr[:], cr_out[:])
    CrT = _transpose(nc, ps, sp, Cr[:], 6, idn, "crt")

    sc = pp.tile([128, 224], F32, tag="tailsc")

    def scv(a, b):
        return sc[0:1, a:b]

    mo_ps = ps.tile([1, 6], F32, tag="tps")
    nc.tensor.matmul(mo_ps[:], e5, CrT[:], start=True, stop=True)
    _dcp(nc, scv(0, 6), mo_ps[:])              # side1 moments (tilde)
    mo_ps2 = ps.tile([1, 6], F32, tag="tps")
    nc.tensor.matmul(mo_ps2[:], e5, Cr[:], start=True, stop=True)
    _dcp(nc, scv(6, 12), mo_ps2[:])            # side2 moments (tilde)

    def pair(k):  # element k of each side: free idxs (k, k+6)
        return sc[0:1, 0:12].rearrange("p (g d) -> p d g", g=2)[:, k, :]

    Sxx, Sx, Syy, Sy, Sw = pair(0), pair(2), pair(3), pair(4), pair(5)
    ws = scv(12, 14); nc.vector.tensor_scalar_add(ws, Sw, EPS)
    rws = scv(14, 16); nc.vector.reciprocal(rws, ws)
    cx = scv(16, 18); nc.vector.tensor_tensor(cx, Sx, rws, OP.mult)
    cy = scv(18, 20); nc.vector.tensor_tensor(cy, Sy, rws, OP.mult)
    t_a = scv(20, 22); nc.vector.tensor_tensor(t_a, cx, Sx, OP.mult)
    t_b = scv(22, 24); nc.vector.tensor_tensor(t_b, cy, Sy, OP.mult)
    cdS = scv(24, 26); nc.vector.tensor_tensor(cdS, t_a, t_b, OP.add)
    u_a = scv(26, 28); nc.vector.tensor_tensor(u_a, cx, cx, OP.mult)
    u_b = scv(28, 30); nc.vector.tensor_tensor(u_b, cy, cy, OP.mult)
    c2_ = scv(30, 32); nc.vector.tensor_tensor(c2_, u_a, u_b, OP.add)
    sq_ = scv(32, 34); nc.vector.tensor_tensor(sq_, Sxx, Syy, OP.add)
    n2c = scv(34, 36); nc.vector.tensor_scalar_mul(n2c, cdS, -2.0)
    c2w = scv(36, 38); nc.vector.tensor_tensor(c2w, c2_, Sw, OP.mult)
    m_ = scv(38, 40); nc.vector.tensor_tensor(m_, sq_, n2c, OP.add)
    m2_ = scv(40, 42); nc.vector.tensor_tensor(m2_, m_, c2w, OP.add)
    md2 = scv(42, 44); nc.vector.tensor_tensor(md2, m2_, rws, OP.mult)
    md2e = scv(44, 46); nc.vector.tensor_scalar_add(md2e, md2, EPS)
    md = scv(46, 48); nc.scalar.activation(md, md2e, AF.Sqrt)
    mde = scv(48, 50); nc.vector.tensor_scalar_add(mde, md, EPS)
    rmd = scv(50, 52); nc.vector.reciprocal(rmd, mde)
    s_ = scv(52, 54); nc.vector.tensor_scalar_mul(s_, rmd, SQRT2)

    # L-matrix ingredients (tilde-coord scalars)
    ss = scv(54, 56); nc.vector.tensor_tensor(ss, s_, s_, OP.mult)
    sscx = scv(56, 58); nc.vector.tensor_tensor(sscx, ss, cx, OP.mult)
    sscy = scv(58, 60); nc.vector.tensor_tensor(sscy, ss, cy, OP.mult)
    n2sscx = scv(62, 64); nc.vector.tensor_scalar_mul(n2sscx, sscx, -2.0)
    nsscy = scv(64, 66); nc.vector.tensor_scalar_mul(nsscy, sscy, -1.0)
    nsscx = scv(66, 68); nc.vector.tensor_scalar_mul(nsscx, sscx, -1.0)
    n2sscy = scv(68, 70); nc.vector.tensor_scalar_mul(n2sscy, sscy, -2.0)
    scx = scv(70, 72); nc.vector.tensor_tensor(scx, s_, cx, OP.mult)
    scy = scv(72, 74); nc.vector.tensor_tensor(scy, s_, cy, OP.mult)
    nscx = scv(74, 76); nc.vector.tensor_scalar_mul(nscx, scx, -1.0)
    nscy = scv(76, 78); nc.vector.tensor_scalar_mul(nscy, scy, -1.0)
    sscxcx = scv(78, 80); nc.vector.tensor_tensor(sscxcx, sscx, cx, OP.mult)
    sscxcy = scv(80, 82); nc.vector.tensor_tensor(sscxcy, sscx, cy, OP.mult)
    sscycy = scv(82, 84); nc.vector.tensor_tensor(sscycy, sscy, cy, OP.mult)

    # raw-coord Hartley scalars for the final T1/T2 (x = c0 + x~/s0):
    # s_raw = s0*s~ ; s_raw*cx_raw = s~*(cx~ + s0*c0)
    sr = scv(84, 86); nc.vector.tensor_scalar_mul(sr, s_, S0)
    cxr = scv(86, 88); nc.vector.tensor_scalar_add(cxr, cx, S0 * C0X)
    cyr = scv(88, 90); nc.vector.tensor_scalar_add(cyr, cy, S0 * C0Y)
    u1_ = scv(90, 92); nc.vector.tensor_tensor(u1_, s_, cxr, OP.mult)
    u2_ = scv(92, 94); nc.vector.tensor_tensor(u2_, s_, cyr, OP.mult)
    nscxr = scv(94, 96); nc.vector.tensor_scalar_mul(nscxr, u1_, -1.0)
    nscyr = scv(96, 98); nc.vector.tensor_scalar_mul(nscyr, u2_, -1.0)

    # L^T row vectors for rank-1 assembly: side s base 100+36s, row j at +6j.
    lrows = sc[0:1, 100:172]
    nc.vector.memset(lrows, 0.0)
    lv = lrows.rearrange("p (s k) -> p k s", s=2)  # [1, 36, 2]

    def lwrite(k, src):
        nc.vector.tensor_copy(lv[:, k, :], src)

    lwrite(0, ss)        # row0: [ss, 0, n2sscx, 0, 0, sscxcx]
    lwrite(2, n2sscx)
    lwrite(5, sscxcx)
    lwrite(7, ss)        # row1: [0, ss, nsscy, 0, nsscx, sscxcy]
    lwrite(8, nsscy)
    lwrite(10, nsscx)
    lwrite(11, sscxcy)
    lwrite(14, s_)       # row2: [0, 0, s, 0, 0, nscx]
    lwrite(17, nscx)
    lwrite(21, ss)       # row3: [0, 0, 0, ss, n2sscy, sscycy]
    lwrite(22, n2sscy)
    lwrite(23, sscycy)
    lwrite(28, s_)       # row4: [0, 0, 0, 0, s, nscy]
    lwrite(29, nscy)
    nc.vector.memset(lv[:, 35, :], 1.0)   # row5 = e5

    def lrow(side, j):
        return sc[0:1, 100 + 36 * side + 6 * j:100 + 36 * side + 6 * j + 6]

    # L1T/L2T via rank-1 accumulation: column j of L^T = row j of L
    def build_LT(side, tag):
        lps = ps.tile([6, 6], F32, tag="tps")
        for j in range(6):
            nc.tensor.matmul(lps[:], lrow(side, j), e6row(j),
                             start=(j == 0), stop=(j == 5))
        lt = sp.tile([6, 6], F32, tag=tag)
        _dcp(nc, lt[:], lps[:])
        return lt

    L1Ts = build_LT(0, "L1Ts")
    L2Ts = build_LT(1, "L2Ts")

    # C2^T = L2 @ (L1 @ C)^T
    zps = ps.tile([6, 6], F32, tag="tps")
    nc.tensor.matmul(zps[:], L1Ts[:], Cr[:], start=True, stop=True)   # L1 @ C
    Zs = sp.tile([6, 6], F32, tag="Zs")
    _dcp(nc, Zs[:], zps[:])
    ZTs = _transpose(nc, ps, sp, Zs[:], 6, idn, "zt")
    c2ps = ps.tile([6, 6], F32, tag="tps")
    nc.tensor.matmul(c2ps[:], L2Ts[:], ZTs[:], start=True, stop=True)  # C2^T
    C2Ts = sp.tile([6, 6], F32, tag="C2Ts")
    _dcp(nc, C2Ts[:], c2ps[:])

    # G2 = E C2 E^T : G2[3a+b, 3c+d] = C2[pair(a,b), pair(c,d)]
    z2ps = ps.tile([6, 9], F32, tag="tps")
    nc.tensor.matmul(z2ps[:], C2Ts[:], et69, start=True, stop=True)  # C2 E^T
    Z2s = sp.tile([6, 9], F32, tag="Z2s")
    _dcp(nc, Z2s[:], z2ps[:])
    g_ps = ps.tile([9, 9], F32, tag="tps")
    nc.tensor.matmul(g_ps[:], et69, Z2s[:], start=True, stop=True)    # E @ Z
    G2 = sp.tile([9, 9], F32, tag="G2")
    _dcp(nc, G2[:], g_ps[:])

    # Mmat[3p+q, 3r+s] = G2[3p+r, 3q+s]: bounce via DRAM, 3 row reads
    nc.sync.dma_start(mshuf[:], G2[:])
    Mmat = sp.tile([9, 9], F32, tag="Mmat")
    for p in range(3):
        # Mmat[3p+q, 3r+s] <- mshuf[27p + 9r + 3q + s]; dims (q, r, s)
        nc.sync.dma_start(
            Mmat[3 * p:3 * p + 3, :].rearrange("q (r s) -> q r s", s=3),
            mshuf[:].rearrange("(p q1 r s) -> p q1 r s", p=3, q1=3, r=3)
            .transpose([0, 2, 1, 3])[p])

    # shifted scaled 9x9: Msp = Mmat/(2 lam) - I/2 (sign irrelevant, even pow)
    dg = sp.tile([9, 9], F32, tag="dg")
    nc.vector.tensor_tensor(dg[:], Mmat[:], i9h, OP.mult)  # diag/2
    lam2 = sp.tile([9, 1], F32, tag="lam2")
    nc.vector.tensor_reduce(lam2[:], dg[:], AX.X, OP.add)
    lam2r = sp.tile([9, 1], F32, tag="lam2r")
    nc.gpsimd.partition_all_reduce(lam2r[:], lam2[:], channels=9,
                                   reduce_op=bass_isa.ReduceOp.add)
    lam4 = sp.tile([9, 1], F32, tag="lam4")
    nc.vector.tensor_scalar_mul(lam4[:], lam2r[:], 4.0)  # = 2*lam
    inv2l = sp.tile([9, 1], F32, tag="inv2l")
    nc.vector.reciprocal(inv2l[:], lam4[:])
    Msp = sp.tile([9, 9], F32, tag="Msp")
    nc.vector.scalar_tensor_tensor(Msp[:], Mmat[:], inv2l[:], i9h,
                                   OP.mult, OP.subtract)
    M50 = _pow50(nc, ps, sp, Msp[:], 9, "m9")

    w9ps = ps.tile([1, 9], F32, tag="tps")
    nc.tensor.matmul(w9ps[:], v09, M50[:], start=True, stop=True)
    w9 = sp.tile([1, 9], F32, tag="w9")
    _dcp(nc, w9[:], w9ps[:])
    w9sq = sp.tile([1, 9], F32, tag="w9sq")
    nc.vector.tensor_tensor(w9sq[:], w9[:], w9[:], OP.mult)
    nn9 = sp.tile([1, 1], F32, tag="nn9")
    nc.vector.tensor_reduce(nn9[:], w9sq[:], AX.X, OP.add)
    sr9 = sp.tile([1, 1], F32, tag="sr9")
    nc.scalar.activation(sr9[:], nn9[:], AF.Sqrt)
    rs9 = sp.tile([1, 1], F32, tag="rs9")
    nc.vector.reciprocal(rs9[:], sr9[:])
    v9 = sp.tile([1, 9], F32, tag="v9")
    nc.vector.tensor_tensor(v9[:], w9[:], rs9[:].to_broadcast([1, 9]), OP.mult)

    # Eraw [3,3]: row k = v9[3k:3k+3], via rank-1 matmuls
    erps = ps.tile([3, 3], F32, tag="tps")
    for k in range(3):
        nc.tensor.matmul(erps[:], e3row(k), v9[0:1, 3 * k:3 * k + 3],
                         start=(k == 0), stop=(k == 2))
    Eraw = sp.tile([3, 3], F32, tag="Eraw")
    _dcp(nc, Eraw[:], erps[:])

    # T1m/T2m [3,3] from raw Hartley scalars via rank-1 matmuls.
    # per side 16 slots at 176+16s: buf6 = [sr,0,0,0,sr,0] at +0,
    # col2 = [nscxr, nscyr, 1] at +8.
    tcols = sc[0:1, 176:208]
    nc.vector.memset(tcols, 0.0)
    tcv = tcols.rearrange("p (s k) -> p k s", s=2)  # [1, 16, 2]
    nc.vector.tensor_copy(tcv[:, 0, :], sr)
    nc.vector.tensor_copy(tcv[:, 4, :], sr)
    nc.vector.tensor_copy(tcv[:, 8, :], nscxr)
    nc.vector.tensor_copy(tcv[:, 9, :], nscyr)
    nc.vector.memset(tcv[:, 10, :], 1.0)

    def tcol(side, off, ln):
        return sc[0:1, 176 + 16 * side + off:176 + 16 * side + off + ln]

    def build_T(side, tag):
        tps_ = ps.tile([3, 3], F32, tag="tps")
        nc.tensor.matmul(tps_[:], tcol(side, 0, 3), e3row(0),
                         start=True, stop=False)
        nc.tensor.matmul(tps_[:], tcol(side, 3, 3), e3row(1),
                         start=False, stop=False)
        nc.tensor.matmul(tps_[:], tcol(side, 8, 3), e3row(2),
                         start=False, stop=True)
        tm = sp.tile([3, 3], F32, tag=tag)
        _dcp(nc, tm[:], tps_[:])
        return tm

    T1m = build_T(0, "T1m")
    T2m = build_T(1, "T2m")

    # E = T2^T E_raw T1 (and E^T)
    a1ps = ps.tile([3, 3], F32, tag="tps")
    nc.tensor.matmul(a1ps[:], T2m[:], Eraw[:], start=True, stop=True)
    A1 = sp.tile([3, 3], F32, tag="A1")
    _dcp(nc, A1[:], a1ps[:])
    A1T = _transpose(nc, ps, sp, A1[:], 3, idn, "a1t")
    etps = ps.tile([3, 3], F32, tag="tps")
    nc.tensor.matmul(etps[:], T1m[:], A1T[:], start=True, stop=True)
    ETs = sp.tile([3, 3], F32, tag="ETs")
    _dcp(nc, ETs[:], etps[:])
    Es = _transpose(nc, ps, sp, ETs[:], 3, idn, "es")

    # B = E^T E ; blockdiag 6x6 chain for v1 (max) and v3 (min)
    bps = ps.tile([3, 3], F32, tag="tps")
    nc.tensor.matmul(bps[:], Es[:], Es[:], start=True, stop=True)
    Bm = sp.tile([3, 3], F32, tag="Bm")
    _dcp(nc, Bm[:], bps[:])
    dg3 = sp.tile([3, 3], F32, tag="dg3")
    nc.vector.tensor_tensor(dg3[:], Bm[:], i3c, OP.mult)
    lb = sp.tile([3, 1], F32, tag="lb")
    nc.vector.tensor_reduce(lb[:], dg3[:], AX.X, OP.add)
    lbr = sp.tile([3, 1], F32, tag="lbr")
    nc.gpsimd.partition_all_reduce(lbr[:], lb[:], channels=3,
                                   reduce_op=bass_isa.ReduceOp.add)
    invlb = sp.tile([3, 1], F32, tag="invlb")
    nc.vector.reciprocal(invlb[:], lbr[:])
    Bs3 = sp.tile([3, 3], F32, tag="Bs3")
    nc.vector.tensor_scalar_mul(Bs3[:], Bm[:], invlb[:])
    IB = sp.tile([3, 3], F32, tag="IB")
    nc.vector.tensor_tensor(IB[:], i3c, Bs3[:], OP.subtract)
    bdps = ps.tile([6, 6], F32, tag="tps")
    nc.tensor.matmul(bdps[:, 0:3], sel1, Bs3[:], start=True, stop=True)
    nc.tensor.matmul(bdps[:, 3:6], sel2, IB[:], start=True, stop=True)
    BD = sp.tile([6, 6], F32, tag="BD")
    _dcp(nc, BD[:], bdps[:])
    BD50 = _pow50(nc, ps, sp, BD[:], 6, "m6")

    w6ps = ps.tile([1, 6], F32, tag="tps")
    nc.tensor.matmul(w6ps[:], v06, BD50[:], start=True, stop=True)
    w6 = sp.tile([1, 6], F32, tag="w6")
    _dcp(nc, w6[:], w6ps[:])
    w6sq = sp.tile([1, 6], F32, tag="w6sq")
    nc.vector.tensor_tensor(w6sq[:], w6[:], w6[:], OP.mult)
    nn6 = sp.tile([1, 2], F32, tag="nn6")
    nc.vector.tensor_reduce(nn6[:].unsqueeze(2),
                            w6sq[:].rearrange("p (g d) -> p g d", g=2), AX.X,
                            OP.add)
    sr6 = sp.tile([1, 2], F32, tag="sr6")
    nc.scalar.activation(sr6[:], nn6[:], AF.Sqrt)
    rs6 = sp.tile([1, 2], F32, tag="rs6")
    nc.vector.reciprocal(rs6[:], sr6[:])
    vv = sp.tile([1, 6], F32, tag="vv")
    nc.vector.tensor_tensor(
        vv[:].rearrange("p (g d) -> p g d", g=2),
        w6[:].rearrange("p (g d) -> p g d", g=2),
        rs6[:].unsqueeze(2).to_broadcast([1, 2, 3]), OP.mult)

    # v2 = cross(v3, v1), normalized with EPS (as reference)
    aa = sp.tile([1, 6], F32, tag="aa")
    nc.vector.tensor_copy(
        aa[:].rearrange("p (r d) -> p r d", r=2),
        vv[:, 3:6].unsqueeze(1).to_broadcast([1, 2, 3]))
    bb = sp.tile([1, 6], F32, tag="bb")
    nc.vector.tensor_copy(
        bb[:].rearrange("p (r d) -> p r d", r=2),
        vv[:, 0:3].unsqueeze(1).to_broadcast([1, 2, 3]))
    cr1 = sp.tile([1, 3], F32, tag="cr1")
    nc.vector.tensor_tensor(cr1[:], aa[:, 1:4], bb[:, 2:5], OP.mult)
    cr2 = sp.tile([1, 3], F32, tag="cr2")
    nc.vector.tensor_tensor(cr2[:], aa[:, 2:5], bb[:, 1:4], OP.mult)
    v2r = sp.tile([1, 3], F32, tag="v2r")
    nc.vector.tensor_tensor(v2r[:], cr1[:], cr2[:], OP.subtract)
    v2sq = sp.tile([1, 3], F32, tag="v2sq")
    nc.vector.tensor_tensor(v2sq[:], v2r[:], v2r[:], OP.mult)
    nn2 = sp.tile([1, 1], F32, tag="nn2")
    nc.vector.tensor_reduce(nn2[:], v2sq[:], AX.X, OP.add)
    sr2 = sp.tile([1, 1], F32, tag="sr2")
    nc.scalar.activation(sr2[:], nn2[:], AF.Sqrt)
    sr2e = sp.tile([1, 1], F32, tag="sr2e")
    nc.vector.tensor_scalar_add(sr2e[:], sr2[:], EPS)
    rs2 = sp.tile([1, 1], F32, tag="rs2")
    nc.vector.reciprocal(rs2[:], sr2e[:])
    v2 = sp.tile([1, 3], F32, tag="v2")
    nc.vector.tensor_tensor(v2[:], v2r[:], rs2[:].to_broadcast([1, 3]), OP.mult)

    # Vr [2,3] (rows v1, v2) and Vc [3,2] (cols v1, v2) via rank-1 matmuls
    vrps = ps.tile([2, 3], F32, tag="tps")
    nc.tensor.matmul(vrps[:], e2row(0), vv[:, 0:3], start=True, stop=False)
    nc.tensor.matmul(vrps[:], e2row(1), v2[:], start=False, stop=True)
    Vr = sp.tile([2, 3], F32, tag="Vr")
    _dcp(nc, Vr[:], vrps[:])
    vcps = ps.tile([3, 2], F32, tag="tps")
    nc.tensor.matmul(vcps[:], vv[:, 0:3], e2row(0), start=True, stop=False)
    nc.tensor.matmul(vcps[:], v2[:], e2row(1), start=False, stop=True)
    Vc = sp.tile([3, 2], F32, tag="Vc")
    _dcp(nc, Vc[:], vcps[:])

    evps = ps.tile([2, 3], F32, tag="tps")
    nc.tensor.matmul(evps[:], Vc[:], ETs[:], start=True, stop=True)
    Evr = sp.tile([2, 3], F32, tag="Evr")
    _dcp(nc, Evr[:], evps[:])
    evsq = sp.tile([2, 3], F32, tag="evsq")
    nc.vector.tensor_tensor(evsq[:], Evr[:], Evr[:], OP.mult)
    ss2 = sp.tile([2, 1], F32, tag="ss2")
    nc.vector.tensor_reduce(ss2[:], evsq[:], AX.X, OP.add)
    sv = sp.tile([2, 1], F32, tag="sv")
    nc.scalar.activation(sv[:], ss2[:], AF.Sqrt)
    ssum = sp.tile([2, 1], F32, tag="ssum")
    nc.gpsimd.partition_all_reduce(ssum[:], sv[:], channels=2,
                                   reduce_op=bass_isa.ReduceOp.add)
    savg = sp.tile([2, 1], F32, tag="savg")
    nc.vector.tensor_scalar_mul(savg[:], ssum[:], 0.5)
    sve = sp.tile([2, 1], F32, tag="sve")
    nc.vector.tensor_scalar_add(sve[:], sv[:], EPS)
    rsv = sp.tile([2, 1], F32, tag="rsv")
    nc.vector.reciprocal(rsv[:], sve[:])
    f2 = sp.tile([2, 1], F32, tag="f2")
    nc.vector.tensor_tensor(f2[:], rsv[:], savg[:], OP.mult)
    U2 = sp.tile([2, 3], F32, tag="U2")
    nc.vector.tensor_scalar_mul(U2[:], Evr[:], f2[:])
    ops_ = ps.tile([3, 3], F32, tag="tps")
    nc.tensor.matmul(ops_[:], U2[:], Vr[:], start=True, stop=True)
    outs = sp.tile([3, 3], F32, tag="outs")
    _dcp(nc, outs[:], ops_[:])
    nc.sync.dma_start(out_d[:], outs[:])


def make_in_maps(P, K):
    """Host-side shard + constant prep: list of 8 input dicts."""
    P = np.asarray(P, np.float32)
    K = np.asarray(K, np.float32)
    Pc = np.ascontiguousarray(P[:N, :N])
    M, cpack = host_constants(K)
    m2t = _tile128(M, CB)
    ident = np.eye(128, dtype=np.float32)
    in_maps = []
    for k in range(NCORES):
        sh = Pc[k * SH:(k + 1) * SH]
        in_maps.append({
            "xin": _tile128(sh, RT),
            "m1s": _tile128(M[k * SH:(k + 1) * SH], RT),
            "m2t": m2t,
            "ident": ident,
            "cpack": cpack,
        })
    return in_maps


_NC_CACHE = {}


def kernel(P, K):
    from concourse.bass_utils import run_bass_kernel_spmd
    if "nc" not in _NC_CACHE:
        _NC_CACHE["nc"] = build_nc()
    nc = _NC_CACHE["nc"]
    in_maps = make_in_maps(P, K)
    res = run_bass_kernel_spmd(nc, in_maps, core_ids=list(range(NCORES)))
    return np.asarray(res.results[0]["out"], np.float32)


# revision 41
# speedup vs baseline: 1.0821x; 1.0590x over previous
"""Trainium2 Bass kernel for nn_EssentialMatrixEstimator.

Distribution: data-parallel over the N=3072 rows of Pc across 8 cores
(384 rows each).

Math: the (N*M, 9) epipolar design-matrix Gram collapses to a single 6x6
monomial Gram C = M1^T W M2 computed in HOST-pre-centered coordinates
x~ = s0*(x - c0) (no cancellation).  The Hartley normalization is a 6x6
linear map on monomials, so the normalized Gram is C2 = L1 C L2^T with
L1/L2 built on-device from the Hartley scalars (which live in row/col 5
of C).  Mmat (9x9) is then a pure index expansion of C2.  One AllGather
(column top-3 partials) + one AllReduce (6x6 Gram) total.
W is the bidirectional-top3 (+ >0.01) masked score matrix; exact top-3
via Max8 (column partials read straight from the transpose PSUM, so the
AllGather posts as soon as the input DMA drains).  Row masking is a
single fused scalar_tensor_tensor pass in row-layout during the gather;
masked data is re-transposed during the same window.  Gram matmuls run
as fp32r with the 6-wide monomial matrix PE-stationary.  The 50-step
power iterations run as rescaled repeated squaring (M <- 2*(M@M)).
Small 3x3/2x3 matrices are assembled from partition-0 scalars via PE
rank-1 (outer-product) matmuls instead of DRAM staging round trips.
"""

import os

os.environ.setdefault("JAX_PLATFORMS", "axon")

import numpy as np

import concourse.bass as bass
import concourse.bass_isa as bass_isa
import concourse.mybir as mybir
import concourse.bacc as bacc
import concourse.tile as tile

NCORES = 8
N = 3072
SH = N // NCORES          # 384 rows per core
RT = SH // 128            # 3 row tiles per core
CB = N // 128             # 24 column blocks
F32 = mybir.dt.float32
F32R = mybir.dt.float32r
AF = mybir.ActivationFunctionType
OP = mybir.AluOpType
AX = mybir.AxisListType

EPS = 1e-8
SQRT2 = 1.4142135623730951
INV_SQRT3 = 1.0 / 1.7320508075688772
T0 = float(np.nextafter(np.float32(0.01), np.float32(1)))  # x > 0.01 == x >= T0
H, W = 64, 64

# host pre-centering of the camera-plane grid coordinates
S0 = 20.0
C0X = -0.577
C0Y = -0.432

# colmask runs entirely on DVE: the Pool/gpsimd engine supports neither
# TensorScalarPtr nor broadcast (stride-0) TensorTensor operands
DVE_CM = CB

# cpack const layout (tensor [9, 48]): column ranges
C_I9H = 0      # I9 * 0.5          [9, 9]
C_ET69 = 9     # E^T selector      [6, 9]
C_I3 = 18      # I3                [3, 3]
C_V09 = 21     # full(1/3)         [9, 1]
C_V06 = 22     # full(1/sqrt3)     [6, 1]
C_SEL1 = 23    # [I3 | 0]          [3, 6]
C_SEL2 = 29    # [0 | I3]          [3, 6]
C_E5 = 35      # e5 selector       [6, 1]
C_I6 = 36      # I6                [6, 6]
C_E6F = 48     # flat I6 rows      [1, 36] (row j at 6j, partition 0)
C_E3F = 84     # flat I3 rows      [1, 9]
C_E2F = 93     # flat I2 rows      [1, 4]

PAIRS = [(0, 0), (0, 1), (0, 2), (1, 1), (1, 2), (2, 2)]


def _pidx():
    d = {}
    for i, (a, b) in enumerate(PAIRS):
        d[(a, b)] = i
        d[(b, a)] = i
    return d


def host_constants(K):
    """Pre-centered monomial matrix + packed tail constants (all f32)."""
    idx = np.arange(H * W, dtype=np.float32)
    pix = np.stack([idx % np.float32(W), np.floor(idx / np.float32(W))], -1)
    K_inv = np.linalg.inv(np.asarray(K, np.float32)).astype(np.float32)
    p1h = np.concatenate([pix[:N], np.ones((N, 1), np.float32)], -1)
    pts = (p1h @ K_inv.T)[:, :2].astype(np.float32)  # same grid both sides
    x = (np.float32(S0) * (pts[:, 0] - np.float32(C0X))).astype(np.float32)
    y = (np.float32(S0) * (pts[:, 1] - np.float32(C0Y))).astype(np.float32)
    M = np.stack([x * x, x * y, x, y * y, y, np.ones_like(x)], -1).astype(np.float32)

    cpack = np.zeros((9, 100), np.float32)
    cpack[:9, C_I9H:C_I9H + 9] = 0.5 * np.eye(9, dtype=np.float32)
    pid = _pidx()
    for a in range(3):
        for b in range(3):
            cpack[pid[(a, b)], C_ET69 + 3 * a + b] = 1.0  # ET69[m, 3a+b]
    cpack[:3, C_I3:C_I3 + 3] = np.eye(3, dtype=np.float32)
    cpack[:9, C_V09] = 1.0 / 3.0
    cpack[:6, C_V06] = INV_SQRT3
    cpack[:3, C_SEL1:C_SEL1 + 3] = np.eye(3, dtype=np.float32)
    cpack[:3, C_SEL2 + 3:C_SEL2 + 6] = np.eye(3, dtype=np.float32)
    cpack[5, C_E5] = 1.0
    cpack[:6, C_I6:C_I6 + 6] = np.eye(6, dtype=np.float32)
    cpack[0, C_E6F:C_E6F + 36] = np.eye(6, dtype=np.float32).reshape(-1)
    cpack[0, C_E3F:C_E3F + 9] = np.eye(3, dtype=np.float32).reshape(-1)
    cpack[0, C_E2F:C_E2F + 4] = np.eye(2, dtype=np.float32).reshape(-1)
    return M, cpack


def _tile128(a, ntiles):
    """[ntiles*128, F] -> [128, ntiles*F] with [p, t*F+f] = a[t*128+p, f]."""
    F = a.shape[1]
    return np.ascontiguousarray(
        a.reshape(ntiles, 128, F).transpose(1, 0, 2).reshape(128, ntiles * F)
    )


def _act_copy(nc, out, in_, scale=1.0):
    nc.scalar.activation(out, in_, AF.Copy, scale=scale)


def _dcp(nc, out, in_, scale=None):
    """Tail copies run on DVE (idle there, lower latency than ACT)."""
    if scale is None:
        nc.vector.tensor_copy(out, in_)
    else:
        nc.vector.tensor_scalar_mul(out, in_, scale)


def build_nc():
    """Build the SPMD 8-core Bass program; returns compiled nc."""
    nc = bacc.Bacc("TRN2", target_bir_lowering=False, debug=False,
                   num_devices=NCORES)

    xin = nc.dram_tensor("xin", [128, RT * N], F32, kind="ExternalInput")
    m1s = nc.dram_tensor("m1s", [128, RT * 6], F32R, kind="ExternalInput")
    m2t = nc.dram_tensor("m2t", [128, CB * 6], F32R, kind="ExternalInput")
    ident = nc.dram_tensor("ident", [128, 128], F32, kind="ExternalInput")
    cpk = nc.dram_tensor("cpack", [9, 100], F32, kind="ExternalInput")
    out_d = nc.dram_tensor("out", [3, 3], F32, kind="ExternalOutput")

    cp_in = nc.dram_tensor("cp_in", [128, CB * 3], F32)
    cp_out = nc.dram_tensor("cp_out", [NCORES * 128, CB * 3], F32,
                            addr_space="Shared")
    cr_in = nc.dram_tensor("cr_in", [6, 6], F32)
    cr_out = nc.dram_tensor("cr_out", [6, 6], F32, addr_space="Shared")
    mshuf = nc.dram_tensor("mshuf", [81], F32)

    groups = [list(range(NCORES))]

    with tile.TileContext(nc) as tc:
        with (
            tc.tile_pool(name="persist", bufs=1) as pp,
            tc.tile_pool(name="scratch", bufs=2) as sp,
            tc.tile_pool(name="ps_pt", bufs=3, space="PSUM") as ps1,
            tc.tile_pool(name="ps_w2", bufs=2, space="PSUM") as psw,
            tc.tile_pool(name="ps_tl", bufs=1, space="PSUM") as ps,
            tc.tile_pool(name="ps_acc", bufs=1, space="PSUM") as psa,
            tc.tile_pool(name="ps_c", bufs=1, space="PSUM") as psc,
        ):
            # ---------- P0: loads ----------
            # idn FIRST: every transpose depends on it, and DMA queues drain
            # in issue order.  One dma + one tile per X row tile, so
            # consumers of tile t wait only chunk t.
            idn = pp.tile([128, 128], F32, tag="idn")
            nc.sync.dma_start(idn[:], ident[:])
            cps = pp.tile([9, 100], F32, tag="cpk")
            nc.sync.dma_start(cps[:], cpk[:])
            Xs = []
            for t in range(RT):
                Xi = pp.tile([128, N], F32, tag=f"X{t}")
                nc.sync.dma_start(Xi[:], xin[:, t * N:(t + 1) * N])
                Xs.append(Xi)
            m1t_s = pp.tile([128, RT * 6], F32R, tag="m1")
            nc.sync.dma_start(m1t_s[:], m1s[:])
            m2t_s = pp.tile([128, CB * 6], F32R, tag="m2")
            nc.sync.dma_start(m2t_s[:], m2t[:])

            def Xt(t):
                return Xs[t][:]

            # ---------- P1: raw transposes -> column top-8 ----------
            # t-grouped so tile-t work starts as soon as chunk t lands.
            # Batched handoffs: 4 transposes land in ONE [128, 512] PSUM
            # tile, ONE strided Scalar copy moves them to SBUF (amortizing
            # the ~270ns per-op fixed cost), then one DVE Max8 [128, 384]
            # per block gives the column top-8 directly.  Emitted under
            # high_priority so the scheduler drains this path first.
            JB = 4                      # blocks per PSUM batch
            c8all = pp.tile([128, CB * 8], F32, tag="c8all")
            xtraw = pp.tile([128, CB * SH], F32, tag="xtraw")
            xtv = xtraw[:].rearrange("p (j r) -> p j r", r=SH)
            r8 = pp.tile([128, RT * 8], F32, tag="r8")
            with tc.high_priority():
                for t in range(RT):
                    for jg in range(CB // JB):
                        ptb = ps1.tile([128, JB * 128], F32, tag="pt")
                        for k in range(JB):
                            j = jg * JB + k
                            nc.tensor.transpose(
                                ptb[:, k * 128:(k + 1) * 128],
                                Xt(t)[:, j * 128:(j + 1) * 128], idn[:])
                        _act_copy(
                            nc,
                            xtv[:, jg * JB:(jg + 1) * JB,
                                t * 128:(t + 1) * 128],
                            ptb[:].rearrange("p (k r) -> p k r", r=128))
                for j in range(CB):
                    nc.vector.max(out=c8all[:, j * 8:j * 8 + 8],
                                  in_=xtraw[:, j * SH:(j + 1) * SH])
                c3all = pp.tile([128, CB * 3], F32, tag="c3all")
                nc.vector.tensor_copy(
                    c3all[:].rearrange("p (j s) -> p j s", s=3),
                    c8all[:].rearrange("p (j s) -> p j s", s=8)[:, :, 0:3])
                nc.sync.dma_start(cp_in[:], c3all[:])

                # ---------- collective 1: AllGather column partials --------
                nc.gpsimd.collective_compute(
                    "AllGather", OP.bypass, replica_groups=groups,
                    ins=[cp_in[:]], outs=[cp_out[:]])

                gath = pp.tile([128, NCORES * CB * 3], F32, tag="gath")
                nc.sync.dma_start(
                    gath[:].rearrange("p (k f) -> p k f", k=NCORES),
                    cp_out[:].rearrange("(k p) f -> p k f", p=128))

            # ---------- P2 (during gather): row mask + masked transposes ---
            # row threshold per row is a per-partition scalar in row layout:
            # X_t <- [X_t >= max(r8_t[2], T0)] * X_t  (one fused DVE pass)
            rth = pp.tile([128, RT], F32, tag="rth")
            XT = pp.tile([128, CB * SH], F32, tag="XT")  # [p=col, (j, r)]
            # de-prioritized so the scheduler never interleaves these big DVE
            # ops into the gather-critical chain above
            xv = XT[:].rearrange("p (j r) -> p j r", r=SH)
            with tc.high_priority(offset=-100000):
                for t in range(RT):
                    nc.vector.max(out=r8[:, t * 8:t * 8 + 8], in_=Xt(t))
                    nc.vector.tensor_scalar_max(rth[:, t:t + 1],
                                                r8[:, t * 8 + 2:t * 8 + 3], T0)
                    nc.vector.scalar_tensor_tensor(
                        Xt(t), Xt(t), rth[:, t:t + 1], Xt(t),
                        OP.is_ge, OP.mult)
                    for jg in range(CB // JB):
                        ptb2 = psw.tile([128, JB * 128], F32, tag="ptw")
                        for k in range(JB):
                            j = jg * JB + k
                            nc.tensor.transpose(
                                ptb2[:, k * 128:(k + 1) * 128],
                                Xt(t)[:, j * 128:(j + 1) * 128], idn[:])
                        _act_copy(
                            nc,
                            xv[:, jg * JB:(jg + 1) * JB,
                               t * 128:(t + 1) * 128],
                            ptb2[:].rearrange("p (k r) -> p k r", r=128))

            # ---------- P3: combine -> exact column thresholds ----------
            cm8 = pp.tile([128, CB * 8], F32, tag="cm8")
            gv = gath[:].rearrange("p (k j s) -> p j k s", k=NCORES, s=3)
            for j in range(CB):
                nc.vector.max(out=cm8[:, j * 8:j * 8 + 8], in_=gv[:, j])

            # ---------- P4+P5: column mask fused with Gram ----------
            # per block j: XTr_j <- [XT_j >= tc_j] * XT_j (tc_j per-partition;
            # compare on exact f32, product rounded to fp32r on write)
            # then PE accumulates Bt[m, r] += m2_j^T @ XTr_j  in fp32r.
            # DVE handles most blocks with a fused scalar_tensor_tensor; the
            # gpsimd (no TensorScalarPtr support) takes the tail blocks with
            # a 2-pass broadcast form.
            XTr = pp.tile([128, CB * SH], F32R, tag="XTr")
            psB = psc.tile([6, SH], F32, tag="psB")
            for j in range(CB):
                nc.vector.scalar_tensor_tensor(
                    XTr[:, j * SH:(j + 1) * SH],
                    XT[:, j * SH:(j + 1) * SH],
                    cm8[:, j * 8 + 2:j * 8 + 3],
                    XT[:, j * SH:(j + 1) * SH],
                    OP.is_ge, OP.mult)
                nc.tensor.matmul(
                    psB[:],
                    m2t_s[:, j * 6:(j + 1) * 6],
                    XTr[:, j * SH:(j + 1) * SH],
                    start=(j == 0), stop=(j == CB - 1))
            Bt = sp.tile([6, SH], F32, tag="Bt")
            _act_copy(nc, Bt[:], psB[:])
            # stage 2: C[a, m] = sum_r M1[r, a] B[r, m]
            Bs = sp.tile([128, RT * 6], F32R, tag="Bs")
            for t in range(RT):
                pb = psa.tile([128, 6], F32, tag="pb")
                nc.tensor.transpose(pb[:], Bt[:, t * 128:(t + 1) * 128],
                                    idn[0:6, 0:6])
                _act_copy(nc, Bs[:, t * 6:(t + 1) * 6], pb[:])
            pc1 = psc.tile([6, 6], F32, tag="psB")  # reuse psB's bank
            for t in range(RT):
                nc.tensor.matmul(pc1[:],
                                 m1t_s[:, t * 6:(t + 1) * 6],
                                 Bs[:, t * 6:(t + 1) * 6],
                                 start=(t == 0), stop=(t == RT - 1))
            Cp = sp.tile([6, 6], F32, tag="Cp")
            _act_copy(nc, Cp[:], pc1[:])
            nc.sync.dma_start(cr_in[:], Cp[:])

            # ---------- collective 2: AllReduce 6x6 Gram ----------
            nc.gpsimd.collective_compute(
                "AllReduce", OP.add, replica_groups=groups,
                ins=[cr_in[:]], outs=[cr_out[:]])

            # ---------- tail ----------
            _tail(nc, pp, sp, ps, cps, idn, cr_out, mshuf, out_d)

    nc.compile()
    return nc


def _transpose(nc, ps, sp, in_sb, n, idn, tag):
    """PE-transpose square [n, n] SBUF -> new SBUF tile."""
    pt = ps.tile([n, n], F32, tag="tps")
    nc.tensor.transpose(pt[:], in_sb, idn[:n, :n])
    ot = sp.tile([n, n], F32, tag=f"ot_{tag}")
    _dcp(nc, ot[:], pt[:])
    return ot


def _pow50(nc, ps, sp, m_sb, n, tag):
    """Direction of M^50 v via rescaled squarings M <- 2*(M@M);
    M50 = 2*((2*(M32@M16)) @ M2). All operands symmetric."""
    powers = {}
    cur = m_sb
    for i in range(1, 6):  # M2, M4, M8, M16, M32
        pm = ps.tile([n, n], F32, tag="tps")
        nc.tensor.matmul(pm[:], cur, cur, start=True, stop=True)
        nxt = sp.tile([n, n], F32, tag=f"pws_{tag}_{i}")
        _dcp(nc, nxt[:], pm[:], scale=2.0)
        powers[2 ** i] = nxt
        cur = nxt[:]
    pm = ps.tile([n, n], F32, tag="tps")
    nc.tensor.matmul(pm[:], powers[32][:], powers[16][:], start=True, stop=True)
    m48 = sp.tile([n, n], F32, tag=f"pws_{tag}_48")
    _dcp(nc, m48[:], pm[:], scale=2.0)
    pm = ps.tile([n, n], F32, tag="tps")
    nc.tensor.matmul(pm[:], m48[:], powers[2][:], start=True, stop=True)
    m50 = sp.tile([n, n], F32, tag=f"pws_{tag}_50")
    _dcp(nc, m50[:], pm[:], scale=2.0)
    return m50


def _tail(nc, pp, sp, ps, cps, idn, cr_out, mshuf, out_d):
    """Hartley scalars, L-transform to C2, Mmat, power chains, projection."""
    i9h = cps[0:9, C_I9H:C_I9H + 9]
    et69 = cps[0:6, C_ET69:C_ET69 + 9]
    i3c = cps[0:3, C_I3:C_I3 + 3]
    v09 = cps[0:9, C_V09:C_V09 + 1]
    v06 = cps[0:6, C_V06:C_V06 + 1]
    sel1 = cps[0:3, C_SEL1:C_SEL1 + 6]
    sel2 = cps[0:3, C_SEL2:C_SEL2 + 6]
    e5 = cps[0:6, C_E5:C_E5 + 1]

    def e6row(j):  # I6 row j as [1, 6] on partition 0
        return cps[0:1, C_E6F + 6 * j:C_E6F + 6 * j + 6]

    def e3row(k):  # I3 row k as [1, 3] on partition 0
        return cps[0:1, C_E3F + 3 * k:C_E3F + 3 * k + 3]

    def e2row(k):  # I2 row k as [1, 2] on partition 0
        return cps[0:1, C_E2F + 2 * k:C_E2F + 2 * k + 2]

    Cr = sp.tile([6, 6], F32, tag="Cr")
    nc.sync.dma_start(Cr[:], cr_out[:])
    CrT = _transpose(nc, ps, sp, Cr[:], 6, idn, "crt")

    sc = pp.tile([128, 224], F32, tag="tailsc")

    def scv(a, b):
        return sc[0:1, a:b]

    mo_ps = ps.tile([1, 6], F32, tag="tps")
    nc.tensor.matmul(mo_ps[:], e5, CrT[:], start=True, stop=True)
    _dcp(nc, scv(0, 6), mo_ps[:])              # side1 moments (tilde)
    mo_ps2 = ps.tile([1, 6], F32, tag="tps")
    nc.tensor.matmul(mo_ps2[:], e5, Cr[:], start=True, stop=True)
    _dcp(nc, scv(6, 12), mo_ps2[:])            # side2 moments (tilde)

    def pair(k):  # element k of each side: free idxs (k, k+6)
        return sc[0:1, 0:12].rearrange("p (g d) -> p d g", g=2)[:, k, :]

    Sxx, Sx, Syy, Sy, Sw = pair(0), pair(2), pair(3), pair(4), pair(5)
    ws = scv(12, 14); nc.vector.tensor_scalar_add(ws, Sw, EPS)
    rws = scv(14, 16); nc.vector.reciprocal(rws, ws)
    cx = scv(16, 18); nc.vector.tensor_tensor(cx, Sx, rws, OP.mult)
    cy = scv(18, 20); nc.vector.tensor_tensor(cy, Sy, rws, OP.mult)
    t_a = scv(20, 22); nc.vector.tensor_tensor(t_a, cx, Sx, OP.mult)
    t_b = scv(22, 24); nc.vector.tensor_tensor(t_b, cy, Sy, OP.mult)
    cdS = scv(24, 26); nc.vector.tensor_tensor(cdS, t_a, t_b, OP.add)
    u_a = scv(26, 28); nc.vector.tensor_tensor(u_a, cx, cx, OP.mult)
    u_b = scv(28, 30); nc.vector.tensor_tensor(u_b, cy, cy, OP.mult)
    c2_ = scv(30, 32); nc.vector.tensor_tensor(c2_, u_a, u_b, OP.add)
    sq_ = scv(32, 34); nc.vector.tensor_tensor(sq_, Sxx, Syy, OP.add)
    n2c = scv(34, 36); nc.vector.tensor_scalar_mul(n2c, cdS, -2.0)
    c2w = scv(36, 38); nc.vector.tensor_tensor(c2w, c2_, Sw, OP.mult)
    m_ = scv(38, 40); nc.vector.tensor_tensor(m_, sq_, n2c, OP.add)
    m2_ = scv(40, 42); nc.vector.tensor_tensor(m2_, m_, c2w, OP.add)
    md2 = scv(42, 44); nc.vector.tensor_tensor(md2, m2_, rws, OP.mult)
    md2e = scv(44, 46); nc.vector.tensor_scalar_add(md2e, md2, EPS)
    md = scv(46, 48); nc.scalar.activation(md, md2e, AF.Sqrt)
    mde = scv(48, 50); nc.vector.tensor_scalar_add(mde, md, EPS)
    rmd = scv(50, 52); nc.vector.reciprocal(rmd, mde)
    s_ = scv(52, 54); nc.vector.tensor_scalar_mul(s_, rmd, SQRT2)

    # L-matrix ingredients (tilde-coord scalars)
    ss = scv(54, 56); nc.vector.tensor_tensor(ss, s_, s_, OP.mult)
    sscx = scv(56, 58); nc.vector.tensor_tensor(sscx, ss, cx, OP.mult)
    sscy = scv(58, 60); nc.vector.tensor_tensor(sscy, ss, cy, OP.mult)
    n2sscx = scv(62, 64); nc.vector.tensor_scalar_mul(n2sscx, sscx, -2.0)
    nsscy = scv(64, 66); nc.vector.tensor_scalar_mul(nsscy, sscy, -1.0)
    nsscx = scv(66, 68); nc.vector.tensor_scalar_mul(nsscx, sscx, -1.0)
    n2sscy = scv(68, 70); nc.vector.tensor_scalar_mul(n2sscy, sscy, -2.0)
    scx = scv(70, 72); nc.vector.tensor_tensor(scx, s_, cx, OP.mult)
    scy = scv(72, 74); nc.vector.tensor_tensor(scy, s_, cy, OP.mult)
    nscx = scv(74, 76); nc.vector.tensor_scalar_mul(nscx, scx, -1.0)
    nscy = scv(76, 78); nc.vector.tensor_scalar_mul(nscy, scy, -1.0)
    sscxcx = scv(78, 80); nc.vector.tensor_tensor(sscxcx, sscx, cx, OP.mult)
    sscxcy = scv(80, 82); nc.vector.tensor_tensor(sscxcy, sscx, cy, OP.mult)
    sscycy = scv(82, 84); nc.vector.tensor_tensor(sscycy, sscy, cy, OP.mult)

    # raw-coord Hartley scalars for the final T1/T2 (x = c0 + x~/s0):
    # s_raw = s0*s~ ; s_raw*cx_raw = s~*(cx~ + s0*c0)
    sr = scv(84, 86); nc.vector.tensor_scalar_mul(sr, s_, S0)
    cxr = scv(86, 88); nc.vector.tensor_scalar_add(cxr, cx, S0 * C0X)
    cyr = scv(88, 90); nc.vector.tensor_scalar_add(cyr, cy, S0 * C0Y)
    u1_ = scv(90, 92); nc.vector.tensor_tensor(u1_, s_, cxr, OP.mult)
    u2_ = scv(92, 94); nc.vector.tensor_tensor(u2_, s_, cyr, OP.mult)
    nscxr = scv(94, 96); nc.vector.tensor_scalar_mul(nscxr, u1_, -1.0)
    nscyr = scv(96, 98); nc.vector.tensor_scalar_mul(nscyr, u2_, -1.0)

    # L^T row vectors for rank-1 assembly: side s base 100+36s, row j at +6j.
    lrows = sc[0:1, 100:172]
    nc.vector.memset(lrows, 0.0)
    lv = lrows.rearrange("p (s k) -> p k s", s=2)  # [1, 36, 2]

    def lwrite(k, src):
        nc.vector.tensor_copy(lv[:, k, :], src)

    lwrite(0, ss)        # row0: [ss, 0, n2sscx, 0, 0, sscxcx]
    lwrite(2, n2sscx)
    lwrite(5, sscxcx)
    lwrite(7, ss)        # row1: [0, ss, nsscy, 0, nsscx, sscxcy]
    lwrite(8, nsscy)
    lwrite(10, nsscx)
    lwrite(11, sscxcy)
    lwrite(14, s_)       # row2: [0, 0, s, 0, 0, nscx]
    lwrite(17, nscx)
    lwrite(21, ss)       # row3: [0, 0, 0, ss, n2sscy, sscycy]
    lwrite(22, n2sscy)
    lwrite(23, sscycy)
    lwrite(28, s_)       # row4: [0, 0, 0, 0, s, nscy]
    lwrite(29, nscy)
    nc.vector.memset(lv[:, 35, :], 1.0)   # row5 = e5

    def lrow(side, j):
        return sc[0:1, 100 + 36 * side + 6 * j:100 + 36 * side + 6 * j + 6]

    # L1T/L2T via rank-1 accumulation: column j of L^T = row j of L
    def build_LT(side, tag):
        lps = ps.tile([6, 6], F32, tag="tps")
        for j in range(6):
            nc.tensor.matmul(lps[:], lrow(side, j), e6row(j),
                             start=(j == 0), stop=(j == 5))
        lt = sp.tile([6, 6], F32, tag=tag)
        _dcp(nc, lt[:], lps[:])
        return lt

    L1Ts = build_LT(0, "L1Ts")
    L2Ts = build_LT(1, "L2Ts")

    # C2^T = L2 @ (L1 @ C)^T
    zps = ps.tile([6, 6], F32, tag="tps")
    nc.tensor.matmul(zps[:], L1Ts[:], Cr[:], start=True, stop=True)   # L1 @ C
    Zs = sp.tile([6, 6], F32, tag="Zs")
    _dcp(nc, Zs[:], zps[:])
    ZTs = _transpose(nc, ps, sp, Zs[:], 6, idn, "zt")
    c2ps = ps.tile([6, 6], F32, tag="tps")
    nc.tensor.matmul(c2ps[:], L2Ts[:], ZTs[:], start=True, stop=True)  # C2^T
    C2Ts = sp.tile([6, 6], F32, tag="C2Ts")
    _dcp(nc, C2Ts[:], c2ps[:])

    # G2 = E C2 E^T : G2[3a+b, 3c+d] = C2[pair(a,b), pair(c,d)]
    z2ps = ps.tile([6, 9], F32, tag="tps")
    nc.tensor.matmul(z2ps[:], C2Ts[:], et69, start=True, stop=True)  # C2 E^T
    Z2s = sp.tile([6, 9], F32, tag="Z2s")
    _dcp(nc, Z2s[:], z2ps[:])
    g_ps = ps.tile([9, 9], F32, tag="tps")
    nc.tensor.matmul(g_ps[:], et69, Z2s[:], start=True, stop=True)    # E @ Z
    G2 = sp.tile([9, 9], F32, tag="G2")
    _dcp(nc, G2[:], g_ps[:])

    # Mmat[3p+q, 3r+s] = G2[3p+r, 3q+s]: bounce via DRAM, 3 row reads
    nc.sync.dma_start(mshuf[:], G2[:])
    Mmat = sp.tile([9, 9], F32, tag="Mmat")
    for p in range(3):
        # Mmat[3p+q, 3r+s] <- mshuf[27p + 9r + 3q + s]; dims (q, r, s)
        nc.sync.dma_start(
            Mmat[3 * p:3 * p + 3, :].rearrange("q (r s) -> q r s", s=3),
            mshuf[:].rearrange("(p q1 r s) -> p q1 r s", p=3, q1=3, r=3)
            .transpose([0, 2, 1, 3])[p])

    # shifted scaled 9x9: Msp = Mmat/(2 lam) - I/2 (sign irrelevant, even pow)
    dg = sp.tile([9, 9], F32, tag="dg")
    nc.vector.tensor_tensor(dg[:], Mmat[:], i9h, OP.mult)  # diag/2
    lam2 = sp.tile([9, 1], F32, tag="lam2")
    nc.vector.tensor_reduce(lam2[:], dg[:], AX.X, OP.add)
    lam2r = sp.tile([9, 1], F32, tag="lam2r")
    nc.gpsimd.partition_all_reduce(lam2r[:], lam2[:], channels=9,
                                   reduce_op=bass_isa.ReduceOp.add)
    lam4 = sp.tile([9, 1], F32, tag="lam4")
    nc.vector.tensor_scalar_mul(lam4[:], lam2r[:], 4.0)  # = 2*lam
    inv2l = sp.tile([9, 1], F32, tag="inv2l")
    nc.vector.reciprocal(inv2l[:], lam4[:])
    Msp = sp.tile([9, 9], F32, tag="Msp")
    nc.vector.scalar_tensor_tensor(Msp[:], Mmat[:], inv2l[:], i9h,
                                   OP.mult, OP.subtract)
    M50 = _pow50(nc, ps, sp, Msp[:], 9, "m9")

    w9ps = ps.tile([1, 9], F32, tag="tps")
    nc.tensor.matmul(w9ps[:], v09, M50[:], start=True, stop=True)
    w9 = sp.tile([1, 9], F32, tag="w9")
    _dcp(nc, w9[:], w9ps[:])
    w9sq = sp.tile([1, 9], F32, tag="w9sq")
    nc.vector.tensor_tensor(w9sq[:], w9[:], w9[:], OP.mult)
    nn9 = sp.tile([1, 1], F32, tag="nn9")
    nc.vector.tensor_reduce(nn9[:], w9sq[:], AX.X, OP.add)
    sr9 = sp.tile([1, 1], F32, tag="sr9")
    nc.scalar.activation(sr9[:], nn9[:], AF.Sqrt)
    rs9 = sp.tile([1, 1], F32, tag="rs9")
    nc.vector.reciprocal(rs9[:], sr9[:])
    v9 = sp.tile([1, 9], F32, tag="v9")
    nc.vector.tensor_tensor(v9[:], w9[:], rs9[:].to_broadcast([1, 9]), OP.mult)

    # Eraw [3,3]: row k = v9[3k:3k+3], via rank-1 matmuls
    erps = ps.tile([3, 3], F32, tag="tps")
    for k in range(3):
        nc.tensor.matmul(erps[:], e3row(k), v9[0:1, 3 * k:3 * k + 3],
                         start=(k == 0), stop=(k == 2))
    Eraw = sp.tile([3, 3], F32, tag="Eraw")
    _dcp(nc, Eraw[:], erps[:])

    # T1m/T2m [3,3] from raw Hartley scalars via rank-1 matmuls.
    # per side 16 slots at 176+16s: buf6 = [sr,0,0,0,sr,0] at +0,
    # col2 = [nscxr, nscyr, 1] at +8.
    tcols = sc[0:1, 176:208]
    nc.vector.memset(tcols, 0.0)
    tcv = tcols.rearrange("p (s k) -> p k s", s=2)  # [1, 16, 2]
    nc.vector.tensor_copy(tcv[:, 0, :], sr)
    nc.vector.tensor_copy(tcv[:, 4, :], sr)
    nc.vector.tensor_copy(tcv[:, 8, :], nscxr)
    nc.vector.tensor_copy(tcv[:, 9, :], nscyr)
    nc.vector.memset(tcv[:, 10, :], 1.0)

    def tcol(side, off, ln):
        return sc[0:1, 176 + 16 * side + off:176 + 16 * side + off + ln]

    def build_T(side, tag):
        tps_ = ps.tile([3, 3], F32, tag="tps")
        nc.tensor.matmul(tps_[:], tcol(side, 0, 3), e3row(0),
                         start=True, stop=False)
        nc.tensor.matmul(tps_[:], tcol(side, 3, 3), e3row(1),
                         start=False, stop=False)
        nc.tensor.matmul(tps_[:], tcol(side, 8, 3), e3row(2),
                         start=False, stop=True)
        tm = sp.tile([3, 3], F32, tag=tag)
        _dcp(nc, tm[:], tps_[:])
        return tm

    T1m = build_T(0, "T1m")
    T2m = build_T(1, "T2m")

    # E = T2^T E_raw T1 (and E^T)
    a1ps = ps.tile([3, 3], F32, tag="tps")
    nc.tensor.matmul(a1ps[:], T2m[:], Eraw[:], start=True, stop=True)
    A1 = sp.tile([3, 3], F32, tag="A1")
    _dcp(nc, A1[:], a1ps[:])
    A1T = _transpose(nc, ps, sp, A1[:], 3, idn, "a1t")
    etps = ps.tile([3, 3], F32, tag="tps")
    nc.tensor.matmul(etps[:], T1m[:], A1T[:], start=True, stop=True)
    ETs = sp.tile([3, 3], F32, tag="ETs")
    _dcp(nc, ETs[:], etps[:])
    Es = _transpose(nc, ps, sp, ETs[:], 3, idn, "es")

    # B = E^T E ; blockdiag 6x6 chain for v1 (max) and v3 (min)
    bps = ps.tile([3, 3], F32, tag="tps")
    nc.tensor.matmul(bps[:], Es[:], Es[:], start=True, stop=True)
    Bm = sp.tile([3, 3], F32, tag="Bm")
    _dcp(nc, Bm[:], bps[:])
    dg3 = sp.tile([3, 3], F32, tag="dg3")
    nc.vector.tensor_tensor(dg3[:], Bm[:], i3c, OP.mult)
    lb = sp.tile([3, 1], F32, tag="lb")
    nc.vector.tensor_reduce(lb[:], dg3[:], AX.X, OP.add)
    lbr = sp.tile([3, 1], F32, tag="lbr")
    nc.gpsimd.partition_all_reduce(lbr[:], lb[:], channels=3,
                                   reduce_op=bass_isa.ReduceOp.add)
    invlb = sp.tile([3, 1], F32, tag="invlb")
    nc.vector.reciprocal(invlb[:], lbr[:])
    Bs3 = sp.tile([3, 3], F32, tag="Bs3")
    nc.vector.tensor_scalar_mul(Bs3[:], Bm[:], invlb[:])
    IB = sp.tile([3, 3], F32, tag="IB")
    nc.vector.tensor_tensor(IB[:], i3c, Bs3[:], OP.subtract)
    bdps = ps.tile([6, 6], F32, tag="tps")
    nc.tensor.matmul(bdps[:, 0:3], sel1, Bs3[:], start=True, stop=True)
    nc.tensor.matmul(bdps[:, 3:6], sel2, IB[:], start=True, stop=True)
    BD = sp.tile([6, 6], F32, tag="BD")
    _dcp(nc, BD[:], bdps[:])
    BD50 = _pow50(nc, ps, sp, BD[:], 6, "m6")

    w6ps = ps.tile([1, 6], F32, tag="tps")
    nc.tensor.matmul(w6ps[:], v06, BD50[:], start=True, stop=True)
    w6 = sp.tile([1, 6], F32, tag="w6")
    _dcp(nc, w6[:], w6ps[:])
    w6sq = sp.tile([1, 6], F32, tag="w6sq")
    nc.vector.tensor_tensor(w6sq[:], w6[:], w6[:], OP.mult)
    nn6 = sp.tile([1, 2], F32, tag="nn6")
    nc.vector.tensor_reduce(nn6[:].unsqueeze(2),
                            w6sq[:].rearrange("p (g d) -> p g d", g=2), AX.X,
                            OP.add)
    sr6 = sp.tile([1, 2], F32, tag="sr6")
    nc.scalar.activation(sr6[:], nn6[:], AF.Sqrt)
    rs6 = sp.tile([1, 2], F32, tag="rs6")
    nc.vector.reciprocal(rs6[:], sr6[:])
    vv = sp.tile([1, 6], F32, tag="vv")
    nc.vector.tensor_tensor(
        vv[:].rearrange("p (g d) -> p g d", g=2),
        w6[:].rearrange("p (g d) -> p g d", g=2),
        rs6[:].unsqueeze(2).to_broadcast([1, 2, 3]), OP.mult)

    # v2 = cross(v3, v1), normalized with EPS (as reference)
    aa = sp.tile([1, 6], F32, tag="aa")
    nc.vector.tensor_copy(
        aa[:].rearrange("p (r d) -> p r d", r=2),
        vv[:, 3:6].unsqueeze(1).to_broadcast([1, 2, 3]))
    bb = sp.tile([1, 6], F32, tag="bb")
    nc.vector.tensor_copy(
        bb[:].rearrange("p (r d) -> p r d", r=2),
        vv[:, 0:3].unsqueeze(1).to_broadcast([1, 2, 3]))
    cr1 = sp.tile([1, 3], F32, tag="cr1")
    nc.vector.tensor_tensor(cr1[:], aa[:, 1:4], bb[:, 2:5], OP.mult)
    cr2 = sp.tile([1, 3], F32, tag="cr2")
    nc.vector.tensor_tensor(cr2[:], aa[:, 2:5], bb[:, 1:4], OP.mult)
    v2r = sp.tile([1, 3], F32, tag="v2r")
    nc.vector.tensor_tensor(v2r[:], cr1[:], cr2[:], OP.subtract)
    v2sq = sp.tile([1, 3], F32, tag="v2sq")
    nc.vector.tensor_tensor(v2sq[:], v2r[:], v2r[:], OP.mult)
    nn2 = sp.tile([1, 1], F32, tag="nn2")
    nc.vector.tensor_reduce(nn2[:], v2sq[:], AX.X, OP.add)
    sr2 = sp.tile([1, 1], F32, tag="sr2")
    nc.scalar.activation(sr2[:], nn2[:], AF.Sqrt)
    sr2e = sp.tile([1, 1], F32, tag="sr2e")
    nc.vector.tensor_scalar_add(sr2e[:], sr2[:], EPS)
    rs2 = sp.tile([1, 1], F32, tag="rs2")
    nc.vector.reciprocal(rs2[:], sr2e[:])
    v2 = sp.tile([1, 3], F32, tag="v2")
    nc.vector.tensor_tensor(v2[:], v2r[:], rs2[:].to_broadcast([1, 3]), OP.mult)

    # Vr [2,3] (rows v1, v2) and Vc [3,2] (cols v1, v2) via rank-1 matmuls
    vrps = ps.tile([2, 3], F32, tag="tps")
    nc.tensor.matmul(vrps[:], e2row(0), vv[:, 0:3], start=True, stop=False)
    nc.tensor.matmul(vrps[:], e2row(1), v2[:], start=False, stop=True)
    Vr = sp.tile([2, 3], F32, tag="Vr")
    _dcp(nc, Vr[:], vrps[:])
    vcps = ps.tile([3, 2], F32, tag="tps")
    nc.tensor.matmul(vcps[:], vv[:, 0:3], e2row(0), start=True, stop=False)
    nc.tensor.matmul(vcps[:], v2[:], e2row(1), start=False, stop=True)
    Vc = sp.tile([3, 2], F32, tag="Vc")
    _dcp(nc, Vc[:], vcps[:])

    evps = ps.tile([2, 3], F32, tag="tps")
    nc.tensor.matmul(evps[:], Vc[:], ETs[:], start=True, stop=True)
    Evr = sp.tile([2, 3], F32, tag="Evr")
    _dcp(nc, Evr[:], evps[:])
    evsq = sp.tile([2, 3], F32, tag="evsq")
    nc.vector.tensor_tensor(evsq[:], Evr[:], Evr[:], OP.mult)
    ss2 = sp.tile([2, 1], F32, tag="ss2")
    nc.vector.tensor_reduce(ss2[:], evsq[:], AX.X, OP.add)
    sv = sp.tile([2, 1], F32, tag="sv")
    nc.scalar.activation(sv[:], ss2[:], AF.Sqrt)
    ssum = sp.tile([2, 1], F32, tag="ssum")
    nc.gpsimd.partition_all_reduce(ssum[:], sv[:], channels=2,
                                   reduce_op=bass_isa.ReduceOp.add)
    savg = sp.tile([2, 1], F32, tag="savg")
    nc.vector.tensor_scalar_mul(savg[:], ssum[:], 0.5)
    sve = sp.tile([2, 1], F32, tag="sve")
    nc.vector.tensor_scalar_add(sve[:], sv[:], EPS)
    rsv = sp.tile([2, 1], F32, tag="rsv")
    nc.vector.reciprocal(rsv[:], sve[:])
    f2 = sp.tile([2, 1], F32, tag="f2")
    nc.vector.tensor_tensor(f2[:], rsv[:], savg[:], OP.mult)
    U2 = sp.tile([2, 3], F32, tag="U2")
    nc.vector.tensor_scalar_mul(U2[:], Evr[:], f2[:])
    ops_ = ps.tile([3, 3], F32, tag="tps")
    nc.tensor.matmul(ops_[:], U2[:], Vr[:], start=True, stop=True)
    outs = sp.tile([3, 3], F32, tag="outs")
    _dcp(nc, outs[:], ops_[:])
    nc.sync.dma_start(out_d[:], outs[:])


def make_in_maps(P, K):
    """Host-side shard + constant prep: list of 8 input dicts."""
    P = np.asarray(P, np.float32)
    K = np.asarray(K, np.float32)
    Pc = np.ascontiguousarray(P[:N, :N])
    M, cpack = host_constants(K)
    m2t = _tile128(M, CB)
    ident = np.eye(128, dtype=np.float32)
    in_maps = []
    for k in range(NCORES):
        sh = Pc[k * SH:(k + 1) * SH]
        in_maps.append({
            "xin": _tile128(sh, RT),
            "m1s": _tile128(M[k * SH:(k + 1) * SH], RT),
            "m2t": m2t,
            "ident": ident,
            "cpack": cpack,
        })
    return in_maps


_NC_CACHE = {}


def kernel(P, K):
    from concourse.bass_utils import run_bass_kernel_spmd
    if "nc" not in _NC_CACHE:
        _NC_CACHE["nc"] = build_nc()
    nc = _NC_CACHE["nc"]
    in_maps = make_in_maps(P, K)
    res = run_bass_kernel_spmd(nc, in_maps, core_ids=list(range(NCORES)))
    return np.asarray(res.results[0]["out"], np.float32)


# revision 44
# speedup vs baseline: 1.2482x; 1.1535x over previous
"""Trainium2 Bass kernel for nn_EssentialMatrixEstimator.

Distribution: data-parallel over the N=3072 rows of Pc across 8 cores
(384 rows each).

Math: the (N*M, 9) epipolar design-matrix Gram collapses to a single 6x6
monomial Gram C = M1^T W M2 computed in HOST-pre-centered coordinates
x~ = s0*(x - c0) (no cancellation).  The Hartley normalization is a 6x6
linear map on monomials, so the normalized Gram is C2 = L1 C L2^T with
L1/L2 built on-device from the Hartley scalars (which live in row/col 5
of C).  Mmat (9x9) is then a pure index expansion of C2.  One AllGather
(column top-3 partials) + one AllReduce (6x6 Gram) total.
W is the bidirectional-top3 (+ >0.01) masked score matrix; exact top-3
via Max8 (column partials read straight from the transpose PSUM, so the
AllGather posts as soon as the input DMA drains).  Row masking is a
single fused scalar_tensor_tensor pass in row-layout during the gather;
masked data is re-transposed during the same window.  Gram matmuls run
as fp32r with the 6-wide monomial matrix PE-stationary.  The 50-step
power iterations run as rescaled repeated squaring (M <- 2*(M@M)).
Small 3x3/2x3 matrices are assembled from partition-0 scalars via PE
rank-1 (outer-product) matmuls instead of DRAM staging round trips.
"""

import os

os.environ.setdefault("JAX_PLATFORMS", "axon")

import numpy as np

import concourse.bass as bass
import concourse.bass_isa as bass_isa
import concourse.mybir as mybir
import concourse.bacc as bacc
import concourse.tile as tile

NCORES = 8
N = 3072
SH = N // NCORES          # 384 rows per core
RT = SH // 128            # 3 row tiles per core
CB = N // 128             # 24 column blocks
F32 = mybir.dt.float32
F32R = mybir.dt.float32r
AF = mybir.ActivationFunctionType
OP = mybir.AluOpType
AX = mybir.AxisListType

EPS = 1e-8
SQRT2 = 1.4142135623730951
INV_SQRT3 = 1.0 / 1.7320508075688772
T0 = float(np.nextafter(np.float32(0.01), np.float32(1)))  # x > 0.01 == x >= T0
H, W = 64, 64

# host pre-centering of the camera-plane grid coordinates
S0 = 20.0
C0X = -0.577
C0Y = -0.432

# colmask runs entirely on DVE: the Pool/gpsimd engine supports neither
# TensorScalarPtr nor broadcast (stride-0) TensorTensor operands
DVE_CM = CB

# cpack const layout (tensor [9, 48]): column ranges
C_I9H = 0      # I9 * 0.5          [9, 9]
C_ET69 = 9     # E^T selector      [6, 9]
C_I3 = 18      # I3                [3, 3]
C_V09 = 21     # full(1/3)         [9, 1]
C_V06 = 22     # full(1/sqrt3)     [6, 1]
C_SEL1 = 23    # [I3 | 0]          [3, 6]
C_SEL2 = 29    # [0 | I3]          [3, 6]
C_E5 = 35      # e5 selector       [6, 1]
C_I6 = 36      # I6                [6, 6]
C_E6F = 48     # flat I6 rows      [1, 36] (row j at 6j, partition 0)
C_E3F = 84     # flat I3 rows      [1, 9]
C_E2F = 93     # flat I2 rows      [1, 4]

PAIRS = [(0, 0), (0, 1), (0, 2), (1, 1), (1, 2), (2, 2)]


def _pidx():
    d = {}
    for i, (a, b) in enumerate(PAIRS):
        d[(a, b)] = i
        d[(b, a)] = i
    return d


def host_constants(K):
    """Pre-centered monomial matrix + packed tail constants (all f32)."""
    idx = np.arange(H * W, dtype=np.float32)
    pix = np.stack([idx % np.float32(W), np.floor(idx / np.float32(W))], -1)
    K_inv = np.linalg.inv(np.asarray(K, np.float32)).astype(np.float32)
    p1h = np.concatenate([pix[:N], np.ones((N, 1), np.float32)], -1)
    pts = (p1h @ K_inv.T)[:, :2].astype(np.float32)  # same grid both sides
    x = (np.float32(S0) * (pts[:, 0] - np.float32(C0X))).astype(np.float32)
    y = (np.float32(S0) * (pts[:, 1] - np.float32(C0Y))).astype(np.float32)
    M = np.stack([x * x, x * y, x, y * y, y, np.ones_like(x)], -1).astype(np.float32)

    cpack = np.zeros((9, 100), np.float32)
    cpack[:9, C_I9H:C_I9H + 9] = 0.5 * np.eye(9, dtype=np.float32)
    pid = _pidx()
    for a in range(3):
        for b in range(3):
            cpack[pid[(a, b)], C_ET69 + 3 * a + b] = 1.0  # ET69[m, 3a+b]
    cpack[:3, C_I3:C_I3 + 3] = np.eye(3, dtype=np.float32)
    cpack[:9, C_V09] = 1.0 / 3.0
    cpack[:6, C_V06] = INV_SQRT3
    cpack[:3, C_SEL1:C_SEL1 + 3] = np.eye(3, dtype=np.float32)
    cpack[:3, C_SEL2 + 3:C_SEL2 + 6] = np.eye(3, dtype=np.float32)
    cpack[5, C_E5] = 1.0
    cpack[:6, C_I6:C_I6 + 6] = np.eye(6, dtype=np.float32)
    cpack[0, C_E6F:C_E6F + 36] = np.eye(6, dtype=np.float32).reshape(-1)
    cpack[0, C_E3F:C_E3F + 9] = np.eye(3, dtype=np.float32).reshape(-1)
    cpack[0, C_E2F:C_E2F + 4] = np.eye(2, dtype=np.float32).reshape(-1)
    return M, cpack


def _tile128(a, ntiles):
    """[ntiles*128, F] -> [128, ntiles*F] with [p, t*F+f] = a[t*128+p, f]."""
    F = a.shape[1]
    return np.ascontiguousarray(
        a.reshape(ntiles, 128, F).transpose(1, 0, 2).reshape(128, ntiles * F)
    )


def _act_copy(nc, out, in_, scale=1.0):
    nc.scalar.activation(out, in_, AF.Copy, scale=scale)


def _dcp(nc, out, in_, scale=None):
    """Tail copies run on DVE (idle there, lower latency than ACT)."""
    if scale is None:
        nc.vector.tensor_copy(out, in_)
    else:
        nc.vector.tensor_scalar_mul(out, in_, scale)


def build_nc():
    """Build the SPMD 8-core Bass program; returns compiled nc."""
    nc = bacc.Bacc("TRN2", target_bir_lowering=False, debug=False,
                   num_devices=NCORES)

    xin = nc.dram_tensor("xin", [128, RT * N], F32, kind="ExternalInput")
    m1s = nc.dram_tensor("m1s", [128, RT * 6], F32R, kind="ExternalInput")
    m2t = nc.dram_tensor("m2t", [128, CB * 6], F32R, kind="ExternalInput")
    ident = nc.dram_tensor("ident", [128, 128], F32, kind="ExternalInput")
    cpk = nc.dram_tensor("cpack", [9, 100], F32, kind="ExternalInput")
    out_d = nc.dram_tensor("out", [3, 3], F32, kind="ExternalOutput")

    cp_in = nc.dram_tensor("cp_in", [128, CB * 3], F32)
    cp_out = nc.dram_tensor("cp_out", [NCORES * 128, CB * 3], F32,
                            addr_space="Shared")
    cr_in = nc.dram_tensor("cr_in", [6, 6], F32)
    cr_out = nc.dram_tensor("cr_out", [6, 6], F32, addr_space="Shared")
    mshuf = nc.dram_tensor("mshuf", [81], F32)

    groups = [list(range(NCORES))]

    with tile.TileContext(nc) as tc:
        with (
            tc.tile_pool(name="persist", bufs=1) as pp,
            tc.tile_pool(name="scratch", bufs=2) as sp,
            tc.tile_pool(name="ps_pt", bufs=3, space="PSUM") as ps1,
            tc.tile_pool(name="ps_w2", bufs=2, space="PSUM") as psw,
            tc.tile_pool(name="ps_tl", bufs=1, space="PSUM") as ps,
            tc.tile_pool(name="ps_acc", bufs=1, space="PSUM") as psa,
            tc.tile_pool(name="ps_c", bufs=1, space="PSUM") as psc,
        ):
            # ---------- P0: loads ----------
            # idn FIRST: every transpose depends on it, and DMA queues drain
            # in issue order.  One dma + one tile per X row tile, so
            # consumers of tile t wait only chunk t.
            idn = pp.tile([128, 128], F32, tag="idn")
            nc.sync.dma_start(idn[:], ident[:])
            cps = pp.tile([9, 100], F32, tag="cpk")
            nc.sync.dma_start(cps[:], cpk[:])
            Xs = []
            for t in range(RT):
                Xi = pp.tile([128, N], F32, tag=f"X{t}")
                # two half-column dmas per tile: the final arriving piece
                # leaves only half a tile of transposes before the gather
                nc.sync.dma_start(Xi[:, 0:N // 2], xin[:, t * N:t * N + N // 2])
                nc.sync.dma_start(Xi[:, N // 2:N],
                                  xin[:, t * N + N // 2:(t + 1) * N])
                Xs.append(Xi)
            m1t_s = pp.tile([128, RT * 6], F32R, tag="m1")
            nc.sync.dma_start(m1t_s[:], m1s[:])
            m2t_s = pp.tile([128, CB * 6], F32R, tag="m2")
            nc.sync.dma_start(m2t_s[:], m2t[:])

            def Xt(t):
                return Xs[t][:]

            # ---------- P1: raw transposes -> column top-8 ----------
            # t-grouped so tile-t work starts as soon as chunk t lands.
            # Batched handoffs: 4 transposes land in ONE [128, 512] PSUM
            # tile, ONE strided Scalar copy moves them to SBUF (amortizing
            # the ~270ns per-op fixed cost), then one DVE Max8 [128, 384]
            # per block gives the column top-8 directly.  Emitted under
            # high_priority so the scheduler drains this path first.
            JB = 4                      # blocks per PSUM batch
            c8all = pp.tile([128, CB * 8], F32, tag="c8all")
            xtraw = pp.tile([128, CB * SH], F32, tag="xtraw")
            xtv = xtraw[:].rearrange("p (j r) -> p j r", r=SH)
            r8 = pp.tile([128, RT * 8], F32, tag="r8")
            with tc.high_priority():
                for t in range(RT):
                    for jg in range(CB // JB):
                        ptb = ps1.tile([128, JB * 128], F32, tag="pt")
                        for k in range(JB):
                            j = jg * JB + k
                            nc.tensor.transpose(
                                ptb[:, k * 128:(k + 1) * 128],
                                Xt(t)[:, j * 128:(j + 1) * 128], idn[:])
                        _act_copy(
                            nc,
                            xtv[:, jg * JB:(jg + 1) * JB,
                                t * 128:(t + 1) * 128],
                            ptb[:].rearrange("p (k r) -> p k r", r=128))
                for j in range(CB):
                    nc.vector.max(out=c8all[:, j * 8:j * 8 + 8],
                                  in_=xtraw[:, j * SH:(j + 1) * SH])
                c3all = pp.tile([128, CB * 3], F32, tag="c3all")
                nc.vector.tensor_copy(
                    c3all[:].rearrange("p (j s) -> p j s", s=3),
                    c8all[:].rearrange("p (j s) -> p j s", s=8)[:, :, 0:3])
                nc.sync.dma_start(cp_in[:], c3all[:])

                # ---------- collective 1: AllGather column partials --------
                nc.gpsimd.collective_compute(
                    "AllGather", OP.bypass, replica_groups=groups,
                    ins=[cp_in[:]], outs=[cp_out[:]])

                gath = pp.tile([128, NCORES * CB * 3], F32, tag="gath")
                nc.sync.dma_start(
                    gath[:].rearrange("p (k f) -> p k f", k=NCORES),
                    cp_out[:].rearrange("(k p) f -> p k f", p=128))

            # ---------- P2 (during gather): row mask + masked transposes ---
            # row threshold per row is a per-partition scalar in row layout:
            # X_t <- [X_t >= max(r8_t[2], T0)] * X_t  (one fused DVE pass)
            rth = pp.tile([128, RT], F32, tag="rth")
            XT = pp.tile([128, CB * SH], F32, tag="XT")  # [p=col, (j, r)]
            # de-prioritized so the scheduler never interleaves these big DVE
            # ops into the gather-critical chain above
            xv = XT[:].rearrange("p (j r) -> p j r", r=SH)
            with tc.high_priority(offset=-100000):
                for t in range(RT):
                    nc.vector.max(out=r8[:, t * 8:t * 8 + 8], in_=Xt(t))
                    nc.vector.tensor_scalar_max(rth[:, t:t + 1],
                                                r8[:, t * 8 + 2:t * 8 + 3], T0)
                    nc.vector.scalar_tensor_tensor(
                        Xt(t), Xt(t), rth[:, t:t + 1], Xt(t),
                        OP.is_ge, OP.mult)
                    for jg in range(CB // JB):
                        ptb2 = psw.tile([128, JB * 128], F32, tag="ptw")
                        for k in range(JB):
                            j = jg * JB + k
                            nc.tensor.transpose(
                                ptb2[:, k * 128:(k + 1) * 128],
                                Xt(t)[:, j * 128:(j + 1) * 128], idn[:])
                        _act_copy(
                            nc,
                            xv[:, jg * JB:(jg + 1) * JB,
                               t * 128:(t + 1) * 128],
                            ptb2[:].rearrange("p (k r) -> p k r", r=128))

            # ---------- P3: combine -> exact column thresholds ----------
            cm8 = pp.tile([128, CB * 8], F32, tag="cm8")
            gv = gath[:].rearrange("p (k j s) -> p j k s", k=NCORES, s=3)
            for j in range(CB):
                nc.vector.max(out=cm8[:, j * 8:j * 8 + 8], in_=gv[:, j])

            # ---------- P4+P5: column mask fused with Gram ----------
            # per block j: XTr_j <- [XT_j >= tc_j] * XT_j (tc_j per-partition;
            # compare on exact f32, product rounded to fp32r on write)
            # then PE accumulates Bt[m, r] += m2_j^T @ XTr_j  in fp32r.
            # DVE handles most blocks with a fused scalar_tensor_tensor; the
            # gpsimd (no TensorScalarPtr support) takes the tail blocks with
            # a 2-pass broadcast form.
            XTr = pp.tile([128, CB * SH], F32R, tag="XTr")
            psB = psc.tile([6, SH], F32, tag="psB")
            for j in range(CB):
                nc.vector.scalar_tensor_tensor(
                    XTr[:, j * SH:(j + 1) * SH],
                    XT[:, j * SH:(j + 1) * SH],
                    cm8[:, j * 8 + 2:j * 8 + 3],
                    XT[:, j * SH:(j + 1) * SH],
                    OP.is_ge, OP.mult)
                nc.tensor.matmul(
                    psB[:],
                    m2t_s[:, j * 6:(j + 1) * 6],
                    XTr[:, j * SH:(j + 1) * SH],
                    start=(j == 0), stop=(j == CB - 1))
            Bt = sp.tile([6, SH], F32, tag="Bt")
            _act_copy(nc, Bt[:], psB[:])
            # stage 2: C[a, m] = sum_r M1[r, a] B[r, m]
            Bs = sp.tile([128, RT * 6], F32R, tag="Bs")
            for t in range(RT):
                pb = psa.tile([128, 6], F32, tag="pb")
                nc.tensor.transpose(pb[:], Bt[:, t * 128:(t + 1) * 128],
                                    idn[0:6, 0:6])
                _act_copy(nc, Bs[:, t * 6:(t + 1) * 6], pb[:])
            pc1 = psc.tile([6, 6], F32, tag="psB")  # reuse psB's bank
            for t in range(RT):
                nc.tensor.matmul(pc1[:],
                                 m1t_s[:, t * 6:(t + 1) * 6],
                                 Bs[:, t * 6:(t + 1) * 6],
                                 start=(t == 0), stop=(t == RT - 1))
            Cp = sp.tile([6, 6], F32, tag="Cp")
            _act_copy(nc, Cp[:], pc1[:])
            nc.sync.dma_start(cr_in[:], Cp[:])

            # ---------- collective 2: AllReduce 6x6 Gram ----------
            nc.gpsimd.collective_compute(
                "AllReduce", OP.add, replica_groups=groups,
                ins=[cr_in[:]], outs=[cr_out[:]])

            # ---------- tail ----------
            _tail(nc, pp, sp, ps, cps, idn, cr_out, mshuf, out_d)

    nc.compile()
    return nc


def _transpose(nc, ps, sp, in_sb, n, idn, tag):
    """PE-transpose square [n, n] SBUF -> new SBUF tile."""
    pt = ps.tile([n, n], F32, tag="tps")
    nc.tensor.transpose(pt[:], in_sb, idn[:n, :n])
    ot = sp.tile([n, n], F32, tag=f"ot_{tag}")
    _dcp(nc, ot[:], pt[:])
    return ot


def _pow50(nc, ps, sp, m_sb, n, tag):
    """Direction of M^50 v via rescaled squarings M <- 2*(M@M);
    M50 = 2*((2*(M32@M16)) @ M2). All operands symmetric."""
    powers = {}
    cur = m_sb
    for i in range(1, 6):  # M2, M4, M8, M16, M32
        pm = ps.tile([n, n], F32, tag="tps")
        nc.tensor.matmul(pm[:], cur, cur, start=True, stop=True)
        nxt = sp.tile([n, n], F32, tag=f"pws_{tag}_{i}")
        _dcp(nc, nxt[:], pm[:], scale=2.0)
        powers[2 ** i] = nxt
        cur = nxt[:]
    pm = ps.tile([n, n], F32, tag="tps")
    nc.tensor.matmul(pm[:], powers[32][:], powers[16][:], start=True, stop=True)
    m48 = sp.tile([n, n], F32, tag=f"pws_{tag}_48")
    _dcp(nc, m48[:], pm[:], scale=2.0)
    pm = ps.tile([n, n], F32, tag="tps")
    nc.tensor.matmul(pm[:], m48[:], powers[2][:], start=True, stop=True)
    m50 = sp.tile([n, n], F32, tag=f"pws_{tag}_50")
    _dcp(nc, m50[:], pm[:], scale=2.0)
    return m50


def _tail(nc, pp, sp, ps, cps, idn, cr_out, mshuf, out_d):
    """Hartley scalars, L-transform to C2, Mmat, power chains, projection."""
    i9h = cps[0:9, C_I9H:C_I9H + 9]
    et69 = cps[0:6, C_ET69:C_ET69 + 9]
    i3c = cps[0:3, C_I3:C_I3 + 3]
    v09 = cps[0:9, C_V09:C_V09 + 1]
    v06 = cps[0:6, C_V06:C_V06 + 1]
    sel1 = cps[0:3, C_SEL1:C_SEL1 + 6]
    sel2 = cps[0:3, C_SEL2:C_SEL2 + 6]
    e5 = cps[0:6, C_E5:C_E5 + 1]

    def e6row(j):  # I6 row j as [1, 6] on partition 0
        return cps[0:1, C_E6F + 6 * j:C_E6F + 6 * j + 6]

    def e3row(k):  # I3 row k as [1, 3] on partition 0
        return cps[0:1, C_E3F + 3 * k:C_E3F + 3 * k + 3]

    def e2row(k):  # I2 row k as [1, 2] on partition 0
        return cps[0:1, C_E2F + 2 * k:C_E2F + 2 * k + 2]

    Cr = sp.tile([6, 6], F32, tag="Cr")
    nc.sync.dma_start(Cr[:], cr_out[:])

    sc = pp.tile([128, 224], F32, tag="tailsc")

    def scv(a, b):
        return sc[0:1, a:b]

    # side1 moments = column 5 of C: transpose [6,1] -> [1,6]
    mo_ps = ps.tile([1, 6], F32, tag="tps")
    nc.tensor.transpose(mo_ps[:], Cr[:, 5:6], idn[0:6, 0:6])
    _dcp(nc, scv(0, 6), mo_ps[:])              # side1 moments (tilde)
    mo_ps2 = ps.tile([1, 6], F32, tag="tps")
    nc.tensor.matmul(mo_ps2[:], e5, Cr[:], start=True, stop=True)
    _dcp(nc, scv(6, 12), mo_ps2[:])            # side2 moments (tilde)

    def pair(k):  # element k of each side: free idxs (k, k+6)
        return sc[0:1, 0:12].rearrange("p (g d) -> p d g", g=2)[:, k, :]

    Sxx, Sx, Syy, Sy, Sw = pair(0), pair(2), pair(3), pair(4), pair(5)
    ws = scv(12, 14); nc.vector.tensor_scalar_add(ws, Sw, EPS)
    rws = scv(14, 16); nc.vector.reciprocal(rws, ws)
    cx = scv(16, 18); nc.vector.tensor_tensor(cx, Sx, rws, OP.mult)
    cy = scv(18, 20); nc.vector.tensor_tensor(cy, Sy, rws, OP.mult)
    # md2 = E||p||^2 - ||c||^2  (mathematically equal to the reference's
    # E||p - c||^2; pre-centered coords keep the cancellation mild)
    u_a = scv(26, 28); nc.vector.tensor_tensor(u_a, cx, cx, OP.mult)
    u_b = scv(28, 30); nc.vector.tensor_tensor(u_b, cy, cy, OP.mult)
    c2_ = scv(30, 32); nc.vector.tensor_tensor(c2_, u_a, u_b, OP.add)
    sq_ = scv(32, 34); nc.vector.tensor_tensor(sq_, Sxx, Syy, OP.add)
    m_ = scv(38, 40); nc.vector.tensor_tensor(m_, sq_, rws, OP.mult)
    md2 = scv(42, 44); nc.vector.tensor_tensor(md2, m_, c2_, OP.subtract)
    md2e = scv(44, 46); nc.vector.tensor_scalar_add(md2e, md2, EPS)
    md = scv(46, 48); nc.scalar.activation(md, md2e, AF.Sqrt)
    mde = scv(48, 50); nc.vector.tensor_scalar_add(mde, md, EPS)
    rmd = scv(50, 52); nc.vector.reciprocal(rmd, mde)
    s_ = scv(52, 54); nc.vector.tensor_scalar_mul(s_, rmd, SQRT2)

    # L-matrix ingredients (tilde-coord scalars)
    ss = scv(54, 56); nc.vector.tensor_tensor(ss, s_, s_, OP.mult)
    sscx = scv(56, 58); nc.vector.tensor_tensor(sscx, ss, cx, OP.mult)
    sscy = scv(58, 60); nc.vector.tensor_tensor(sscy, ss, cy, OP.mult)
    n2sscx = scv(62, 64); nc.vector.tensor_scalar_mul(n2sscx, sscx, -2.0)
    nsscy = scv(64, 66); nc.vector.tensor_scalar_mul(nsscy, sscy, -1.0)
    nsscx = scv(66, 68); nc.vector.tensor_scalar_mul(nsscx, sscx, -1.0)
    n2sscy = scv(68, 70); nc.vector.tensor_scalar_mul(n2sscy, sscy, -2.0)
    scx = scv(70, 72); nc.vector.tensor_tensor(scx, s_, cx, OP.mult)
    scy = scv(72, 74); nc.vector.tensor_tensor(scy, s_, cy, OP.mult)
    nscx = scv(74, 76); nc.vector.tensor_scalar_mul(nscx, scx, -1.0)
    nscy = scv(76, 78); nc.vector.tensor_scalar_mul(nscy, scy, -1.0)
    sscxcx = scv(78, 80); nc.vector.tensor_tensor(sscxcx, sscx, cx, OP.mult)
    sscxcy = scv(80, 82); nc.vector.tensor_tensor(sscxcy, sscx, cy, OP.mult)
    sscycy = scv(82, 84); nc.vector.tensor_tensor(sscycy, sscy, cy, OP.mult)

    # raw-coord Hartley scalars for the final T1/T2 (x = c0 + x~/s0):
    # s_raw = s0*s~ ; s_raw*cx_raw = s~*(cx~ + s0*c0)
    sr = scv(84, 86); nc.vector.tensor_scalar_mul(sr, s_, S0)
    cxr = scv(86, 88); nc.vector.tensor_scalar_add(cxr, cx, S0 * C0X)
    cyr = scv(88, 90); nc.vector.tensor_scalar_add(cyr, cy, S0 * C0Y)
    u1_ = scv(90, 92); nc.vector.tensor_tensor(u1_, s_, cxr, OP.mult)
    u2_ = scv(92, 94); nc.vector.tensor_tensor(u2_, s_, cyr, OP.mult)
    nscxr = scv(94, 96); nc.vector.tensor_scalar_mul(nscxr, u1_, -1.0)
    nscyr = scv(96, 98); nc.vector.tensor_scalar_mul(nscyr, u2_, -1.0)

    # L^T row vectors for rank-1 assembly: side s base 100+36s, row j at +6j.
    lrows = sc[0:1, 100:172]
    nc.vector.memset(lrows, 0.0)
    lv = lrows.rearrange("p (s k) -> p k s", s=2)  # [1, 36, 2]

    def lwrite(k, src):
        nc.vector.tensor_copy(lv[:, k, :], src)

    lwrite(0, ss)        # row0: [ss, 0, n2sscx, 0, 0, sscxcx]
    lwrite(2, n2sscx)
    lwrite(5, sscxcx)
    lwrite(7, ss)        # row1: [0, ss, nsscy, 0, nsscx, sscxcy]
    lwrite(8, nsscy)
    lwrite(10, nsscx)
    lwrite(11, sscxcy)
    lwrite(14, s_)       # row2: [0, 0, s, 0, 0, nscx]
    lwrite(17, nscx)
    lwrite(21, ss)       # row3: [0, 0, 0, ss, n2sscy, sscycy]
    lwrite(22, n2sscy)
    lwrite(23, sscycy)
    lwrite(28, s_)       # row4: [0, 0, 0, 0, s, nscy]
    lwrite(29, nscy)
    nc.vector.memset(lv[:, 35, :], 1.0)   # row5 = e5

    def lrow(side, j):
        return sc[0:1, 100 + 36 * side + 6 * j:100 + 36 * side + 6 * j + 6]

    # L1T/L2T via rank-1 accumulation: column j of L^T = row j of L
    def build_LT(side, tag):
        lps = ps.tile([6, 6], F32, tag="tps")
        for j in range(6):
            nc.tensor.matmul(lps[:], lrow(side, j), e6row(j),
                             start=(j == 0), stop=(j == 5))
        lt = sp.tile([6, 6], F32, tag=tag)
        _dcp(nc, lt[:], lps[:])
        return lt

    L1Ts = build_LT(0, "L1Ts")
    L2Ts = build_LT(1, "L2Ts")

    # C2^T = L2 @ (L1 @ C)^T
    zps = ps.tile([6, 6], F32, tag="tps")
    nc.tensor.matmul(zps[:], L1Ts[:], Cr[:], start=True, stop=True)   # L1 @ C
    Zs = sp.tile([6, 6], F32, tag="Zs")
    _dcp(nc, Zs[:], zps[:])
    ZTs = _transpose(nc, ps, sp, Zs[:], 6, idn, "zt")
    c2ps = ps.tile([6, 6], F32, tag="tps")
    nc.tensor.matmul(c2ps[:], L2Ts[:], ZTs[:], start=True, stop=True)  # C2^T
    C2Ts = sp.tile([6, 6], F32, tag="C2Ts")
    _dcp(nc, C2Ts[:], c2ps[:])

    # G2 = E C2 E^T : G2[3a+b, 3c+d] = C2[pair(a,b), pair(c,d)]
    z2ps = ps.tile([6, 9], F32, tag="tps")
    nc.tensor.matmul(z2ps[:], C2Ts[:], et69, start=True, stop=True)  # C2 E^T
    Z2s = sp.tile([6, 9], F32, tag="Z2s")
    _dcp(nc, Z2s[:], z2ps[:])
    g_ps = ps.tile([9, 9], F32, tag="tps")
    nc.tensor.matmul(g_ps[:], et69, Z2s[:], start=True, stop=True)    # E @ Z
    G2 = sp.tile([9, 9], F32, tag="G2")
    _dcp(nc, G2[:], g_ps[:])

    # Mmat[3p+q, 3r+s] = G2[3p+r, 3q+s]: bounce via DRAM, 3 row reads
    nc.sync.dma_start(mshuf[:], G2[:])
    Mmat = sp.tile([9, 9], F32, tag="Mmat")
    for p in range(3):
        # Mmat[3p+q, 3r+s] <- mshuf[27p + 9r + 3q + s]; dims (q, r, s)
        nc.sync.dma_start(
            Mmat[3 * p:3 * p + 3, :].rearrange("q (r s) -> q r s", s=3),
            mshuf[:].rearrange("(p q1 r s) -> p q1 r s", p=3, q1=3, r=3)
            .transpose([0, 2, 1, 3])[p])

    # shifted scaled 9x9: Msp = Mmat/(2 lam) - I/2 (sign irrelevant, even pow)
    dg = sp.tile([9, 9], F32, tag="dg")
    nc.vector.tensor_tensor(dg[:], Mmat[:], i9h, OP.mult)  # diag/2
    lam2 = sp.tile([9, 1], F32, tag="lam2")
    nc.vector.tensor_reduce(lam2[:], dg[:], AX.X, OP.add)
    lam2r = sp.tile([9, 1], F32, tag="lam2r")
    nc.gpsimd.partition_all_reduce(lam2r[:], lam2[:], channels=9,
                                   reduce_op=bass_isa.ReduceOp.add)
    lam4 = sp.tile([9, 1], F32, tag="lam4")
    nc.vector.tensor_scalar_mul(lam4[:], lam2r[:], 4.0)  # = 2*lam
    inv2l = sp.tile([9, 1], F32, tag="inv2l")
    nc.vector.reciprocal(inv2l[:], lam4[:])
    Msp = sp.tile([9, 9], F32, tag="Msp")
    nc.vector.scalar_tensor_tensor(Msp[:], Mmat[:], inv2l[:], i9h,
                                   OP.mult, OP.subtract)
    M50 = _pow50(nc, ps, sp, Msp[:], 9, "m9")

    w9ps = ps.tile([1, 9], F32, tag="tps")
    nc.tensor.matmul(w9ps[:], v09, M50[:], start=True, stop=True)
    w9 = sp.tile([1, 9], F32, tag="w9")
    _dcp(nc, w9[:], w9ps[:])
    w9sq = sp.tile([1, 9], F32, tag="w9sq")
    nc.vector.tensor_tensor(w9sq[:], w9[:], w9[:], OP.mult)
    nn9 = sp.tile([1, 1], F32, tag="nn9")
    nc.vector.tensor_reduce(nn9[:], w9sq[:], AX.X, OP.add)
    sr9 = sp.tile([1, 1], F32, tag="sr9")
    nc.scalar.activation(sr9[:], nn9[:], AF.Sqrt)
    rs9 = sp.tile([1, 1], F32, tag="rs9")
    nc.vector.reciprocal(rs9[:], sr9[:])
    v9 = sp.tile([1, 9], F32, tag="v9")
    nc.vector.tensor_tensor(v9[:], w9[:], rs9[:].to_broadcast([1, 9]), OP.mult)

    # Eraw [3,3]: row k = v9[3k:3k+3], via rank-1 matmuls
    erps = ps.tile([3, 3], F32, tag="tps")
    for k in range(3):
        nc.tensor.matmul(erps[:], e3row(k), v9[0:1, 3 * k:3 * k + 3],
                         start=(k == 0), stop=(k == 2))
    Eraw = sp.tile([3, 3], F32, tag="Eraw")
    _dcp(nc, Eraw[:], erps[:])

    # T1m/T2m [3,3] from raw Hartley scalars via rank-1 matmuls.
    # per side 16 slots at 176+16s: buf6 = [sr,0,0,0,sr,0] at +0,
    # col2 = [nscxr, nscyr, 1] at +8.
    tcols = sc[0:1, 176:208]
    nc.vector.memset(tcols, 0.0)
    tcv = tcols.rearrange("p (s k) -> p k s", s=2)  # [1, 16, 2]
    nc.vector.tensor_copy(tcv[:, 0, :], sr)
    nc.vector.tensor_copy(tcv[:, 4, :], sr)
    nc.vector.tensor_copy(tcv[:, 8, :], nscxr)
    nc.vector.tensor_copy(tcv[:, 9, :], nscyr)
    nc.vector.memset(tcv[:, 10, :], 1.0)

    def tcol(side, off, ln):
        return sc[0:1, 176 + 16 * side + off:176 + 16 * side + off + ln]

    def build_T(side, tag):
        tps_ = ps.tile([3, 3], F32, tag="tps")
        nc.tensor.matmul(tps_[:], tcol(side, 0, 3), e3row(0),
                         start=True, stop=False)
        nc.tensor.matmul(tps_[:], tcol(side, 3, 3), e3row(1),
                         start=False, stop=False)
        nc.tensor.matmul(tps_[:], tcol(side, 8, 3), e3row(2),
                         start=False, stop=True)
        tm = sp.tile([3, 3], F32, tag=tag)
        _dcp(nc, tm[:], tps_[:])
        return tm

    T1m = build_T(0, "T1m")
    T2m = build_T(1, "T2m")

    # E = T2^T E_raw T1 (and E^T)
    a1ps = ps.tile([3, 3], F32, tag="tps")
    nc.tensor.matmul(a1ps[:], T2m[:], Eraw[:], start=True, stop=True)
    A1 = sp.tile([3, 3], F32, tag="A1")
    _dcp(nc, A1[:], a1ps[:])
    A1T = _transpose(nc, ps, sp, A1[:], 3, idn, "a1t")
    etps = ps.tile([3, 3], F32, tag="tps")
    nc.tensor.matmul(etps[:], T1m[:], A1T[:], start=True, stop=True)
    ETs = sp.tile([3, 3], F32, tag="ETs")
    _dcp(nc, ETs[:], etps[:])
    Es = _transpose(nc, ps, sp, ETs[:], 3, idn, "es")

    # B = E^T E ; blockdiag 6x6 chain for v1 (max) and v3 (min)
    bps = ps.tile([3, 3], F32, tag="tps")
    nc.tensor.matmul(bps[:], Es[:], Es[:], start=True, stop=True)
    Bm = sp.tile([3, 3], F32, tag="Bm")
    _dcp(nc, Bm[:], bps[:])
    dg3 = sp.tile([3, 3], F32, tag="dg3")
    nc.vector.tensor_tensor(dg3[:], Bm[:], i3c, OP.mult)
    lb = sp.tile([3, 1], F32, tag="lb")
    nc.vector.tensor_reduce(lb[:], dg3[:], AX.X, OP.add)
    lbr = sp.tile([3, 1], F32, tag="lbr")
    nc.gpsimd.partition_all_reduce(lbr[:], lb[:], channels=3,
                                   reduce_op=bass_isa.ReduceOp.add)
    invlb = sp.tile([3, 1], F32, tag="invlb")
    nc.vector.reciprocal(invlb[:], lbr[:])
    Bs3 = sp.tile([3, 3], F32, tag="Bs3")
    nc.vector.tensor_scalar_mul(Bs3[:], Bm[:], invlb[:])
    IB = sp.tile([3, 3], F32, tag="IB")
    nc.vector.tensor_tensor(IB[:], i3c, Bs3[:], OP.subtract)
    bdps = ps.tile([6, 6], F32, tag="tps")
    nc.tensor.matmul(bdps[:, 0:3], sel1, Bs3[:], start=True, stop=True)
    nc.tensor.matmul(bdps[:, 3:6], sel2, IB[:], start=True, stop=True)
    BD = sp.tile([6, 6], F32, tag="BD")
    _dcp(nc, BD[:], bdps[:])
    BD50 = _pow50(nc, ps, sp, BD[:], 6, "m6")

    w6ps = ps.tile([1, 6], F32, tag="tps")
    nc.tensor.matmul(w6ps[:], v06, BD50[:], start=True, stop=True)
    w6 = sp.tile([1, 6], F32, tag="w6")
    _dcp(nc, w6[:], w6ps[:])
    w6sq = sp.tile([1, 6], F32, tag="w6sq")
    nc.vector.tensor_tensor(w6sq[:], w6[:], w6[:], OP.mult)
    nn6 = sp.tile([1, 2], F32, tag="nn6")
    nc.vector.tensor_reduce(nn6[:].unsqueeze(2),
                            w6sq[:].rearrange("p (g d) -> p g d", g=2), AX.X,
                            OP.add)
    sr6 = sp.tile([1, 2], F32, tag="sr6")
    nc.scalar.activation(sr6[:], nn6[:], AF.Sqrt)
    rs6 = sp.tile([1, 2], F32, tag="rs6")
    nc.vector.reciprocal(rs6[:], sr6[:])
    vv = sp.tile([1, 6], F32, tag="vv")
    nc.vector.tensor_tensor(
        vv[:].rearrange("p (g d) -> p g d", g=2),
        w6[:].rearrange("p (g d) -> p g d", g=2),
        rs6[:].unsqueeze(2).to_broadcast([1, 2, 3]), OP.mult)

    # v2 = cross(v3, v1), normalized with EPS (as reference)
    aa = sp.tile([1, 6], F32, tag="aa")
    nc.vector.tensor_copy(
        aa[:].rearrange("p (r d) -> p r d", r=2),
        vv[:, 3:6].unsqueeze(1).to_broadcast([1, 2, 3]))
    bb = sp.tile([1, 6], F32, tag="bb")
    nc.vector.tensor_copy(
        bb[:].rearrange("p (r d) -> p r d", r=2),
        vv[:, 0:3].unsqueeze(1).to_broadcast([1, 2, 3]))
    cr1 = sp.tile([1, 3], F32, tag="cr1")
    nc.vector.tensor_tensor(cr1[:], aa[:, 1:4], bb[:, 2:5], OP.mult)
    cr2 = sp.tile([1, 3], F32, tag="cr2")
    nc.vector.tensor_tensor(cr2[:], aa[:, 2:5], bb[:, 1:4], OP.mult)
    v2r = sp.tile([1, 3], F32, tag="v2r")
    nc.vector.tensor_tensor(v2r[:], cr1[:], cr2[:], OP.subtract)
    v2sq = sp.tile([1, 3], F32, tag="v2sq")
    nc.vector.tensor_tensor(v2sq[:], v2r[:], v2r[:], OP.mult)
    nn2 = sp.tile([1, 1], F32, tag="nn2")
    nc.vector.tensor_reduce(nn2[:], v2sq[:], AX.X, OP.add)
    sr2 = sp.tile([1, 1], F32, tag="sr2")
    nc.scalar.activation(sr2[:], nn2[:], AF.Sqrt)
    sr2e = sp.tile([1, 1], F32, tag="sr2e")
    nc.vector.tensor_scalar_add(sr2e[:], sr2[:], EPS)
    rs2 = sp.tile([1, 1], F32, tag="rs2")
    nc.vector.reciprocal(rs2[:], sr2e[:])
    v2 = sp.tile([1, 3], F32, tag="v2")
    nc.vector.tensor_tensor(v2[:], v2r[:], rs2[:].to_broadcast([1, 3]), OP.mult)

    # Vr [2,3] (rows v1, v2) and Vc [3,2] (cols v1, v2) via rank-1 matmuls
    vrps = ps.tile([2, 3], F32, tag="tps")
    nc.tensor.matmul(vrps[:], e2row(0), vv[:, 0:3], start=True, stop=False)
    nc.tensor.matmul(vrps[:], e2row(1), v2[:], start=False, stop=True)
    Vr = sp.tile([2, 3], F32, tag="Vr")
    _dcp(nc, Vr[:], vrps[:])
    vcps = ps.tile([3, 2], F32, tag="tps")
    nc.tensor.matmul(vcps[:], vv[:, 0:3], e2row(0), start=True, stop=False)
    nc.tensor.matmul(vcps[:], v2[:], e2row(1), start=False, stop=True)
    Vc = sp.tile([3, 2], F32, tag="Vc")
    _dcp(nc, Vc[:], vcps[:])

    evps = ps.tile([2, 3], F32, tag="tps")
    nc.tensor.matmul(evps[:], Vc[:], ETs[:], start=True, stop=True)
    Evr = sp.tile([2, 3], F32, tag="Evr")
    _dcp(nc, Evr[:], evps[:])
    evsq = sp.tile([2, 3], F32, tag="evsq")
    nc.vector.tensor_tensor(evsq[:], Evr[:], Evr[:], OP.mult)
    ss2 = sp.tile([2, 1], F32, tag="ss2")
    nc.vector.tensor_reduce(ss2[:], evsq[:], AX.X, OP.add)
    sv = sp.tile([2, 1], F32, tag="sv")
    nc.scalar.activation(sv[:], ss2[:], AF.Sqrt)
    ssum = sp.tile([2, 1], F32, tag="ssum")
    nc.gpsimd.partition_all_reduce(ssum[:], sv[:], channels=2,
                                   reduce_op=bass_isa.ReduceOp.add)
    savg = sp.tile([2, 1], F32, tag="savg")
    nc.vector.tensor_scalar_mul(savg[:], ssum[:], 0.5)
    sve = sp.tile([2, 1], F32, tag="sve")
    nc.vector.tensor_scalar_add(sve[:], sv[:], EPS)
    rsv = sp.tile([2, 1], F32, tag="rsv")
    nc.vector.reciprocal(rsv[:], sve[:])
    f2 = sp.tile([2, 1], F32, tag="f2")
    nc.vector.tensor_tensor(f2[:], rsv[:], savg[:], OP.mult)
    U2 = sp.tile([2, 3], F32, tag="U2")
    nc.vector.tensor_scalar_mul(U2[:], Evr[:], f2[:])
    ops_ = ps.tile([3, 3], F32, tag="tps")
    nc.tensor.matmul(ops_[:], U2[:], Vr[:], start=True, stop=True)
    outs = sp.tile([3, 3], F32, tag="outs")
    _dcp(nc, outs[:], ops_[:])
    nc.sync.dma_start(out_d[:], outs[:])


def make_in_maps(P, K):
    """Host-side shard + constant prep: list of 8 input dicts."""
    P = np.asarray(P, np.float32)
    K = np.asarray(K, np.float32)
    Pc = np.ascontiguousarray(P[:N, :N])
    M, cpack = host_constants(K)
    m2t = _tile128(M, CB)
    ident = np.eye(128, dtype=np.float32)
    in_maps = []
    for k in range(NCORES):
        sh = Pc[k * SH:(k + 1) * SH]
        in_maps.append({
            "xin": _tile128(sh, RT),
            "m1s": _tile128(M[k * SH:(k + 1) * SH], RT),
            "m2t": m2t,
            "ident": ident,
            "cpack": cpack,
        })
    return in_maps


_NC_CACHE = {}


def kernel(P, K):
    from concourse.bass_utils import run_bass_kernel_spmd
    if "nc" not in _NC_CACHE:
        _NC_CACHE["nc"] = build_nc()
    nc = _NC_CACHE["nc"]
    in_maps = make_in_maps(P, K)
    res = run_bass_kernel_spmd(nc, in_maps, core_ids=list(range(NCORES)))
    return np.asarray(res.results[0]["out"], np.float32)
